# revision 6
# baseline (speedup 1.0000x reference)
"""Causal multi-head attention (B=2, S=2048, D=1024, H=16, hd=64) for 8 Trainium2
NeuronCores, returning (output, attn_weights) like torch nn.MultiheadAttention.

Sharding: core c handles batch b=c//4 and 4 heads (c%4)*4..+4 (data+tensor
parallel per the Megatron hint). Each core computes its heads' causal attention
weights (written bf16, upper triangle left to the runtime's zero-init) and a
partial output projection; the host sums partials and adds biases.

Device-side dataflow per head:
  natural side : scores = qT.T @ kT (PE, bf16, K=64) -> +mask on diagonal tile
                 -> exp on ACT with accum_out row-sums -> w = e * (1/Z) on DVE
                 -> DMA causal rows to HBM
  transposed   : scoresT = kT_aug.T @ qT_aug with two extra contraction rows
                 carrying ones * (-lnZ) (hi + bf16 residual), so exp gives the
                 *normalized* wT directly -> attn_outT += v.T-stationary matmuls
                 -> out-projection partial from attn_outT.
"""

import numpy as np
import ml_dtypes

import concourse.bass as bass
import concourse.mybir as mybir
import concourse.tile as tile
from concourse import bass_utils
from concourse.vector_clock import ScopedClock

BF16 = mybir.dt.bfloat16
F32 = mybir.dt.float32
AF = mybir.ActivationFunctionType

import os
_DEBUG = bool(os.environ.get("KERNEL_DEBUG"))

B, S, D, H = 2, 2048, 1024, 16
HD = D // H            # 64
HPC = 4                # heads per core
EC = HPC * HD          # 256 e-dims per core
NCORES = 8
NQT = S // 128         # 16 q row-tiles
MASK_VAL = -30000.0


# ---------------------------------------------------------------------------
# Container-walrus workaround: CTRL/Drain instructions only support one
# sync-wait slot; Tile's kernel-tail drain carries one wait per DMA-HW queue.
# Split extras onto single-wait NOPs before the all-engine barrier.
# ---------------------------------------------------------------------------
_patched = False


def _patch_tile_drain():
    global _patched
    if _patched:
        return
    _patched = True

    orig_add = tile.TileContext._add_instruction

    def _add_instruction(self, inst):
        si = getattr(inst, "sync_info", None)
        eng = getattr(inst, "engine", None)
        if (
            si is not None
            and si.on_wait
            and len(si.on_wait) > 1
            and eng is not None
            and eng != mybir.EngineType.Unassigned
        ):
            waits = list(si.on_wait)
            si.on_wait = [waits[-1]]
            nc = self.nc
            for w in waits[:-1]:
                nop = mybir.InstNoOp(
                    name=f"I-waitsplit-{nc.next_id()}",
                    sync_info=mybir.SyncInfo(on_wait=[w], on_update=[]),
                    engine=eng,
                    bass_nofuse=True,
                )
                orig_add(self, nop)
        orig_add(self, inst)

    tile.TileContext._add_instruction = _add_instruction

    def _drain_and_barrier(self, tick_clock, wait_clock):
        nc = self.nc
        drain_inst = nc.sync.drain()
        wait_clock.add_sem_waits(
            drain_inst.ins, ScopedClock({None: tick_clock.global_clock})
        )
        si = drain_inst.ins.sync_info
        if si is not None and si.on_wait and len(si.on_wait) > 1:
            waits = list(si.on_wait)
            si.on_wait = [waits[0]]
            for w in waits[1:]:
                nop = mybir.InstNoOp(
                    name=f"I-waitsplit-{nc.next_id()}",
                    sync_info=mybir.SyncInfo(on_wait=[w], on_update=[]),
                    engine=drain_inst.ins.engine,
                    bass_nofuse=True,
                )
                nc.register_instruction(nop, overwrite=True)
                nc.cur_bb.bb.add_instruction(nop)

        nc.all_engine_barrier()
        assert self.sems is not None
        popped = nc._tile_sem_poison_stack.pop()
        assert popped is self._sem_poison
        nc.clear_and_free_semaphores(list(self.sems.allocated().values()))
        nc.all_engine_barrier()

    tile.TileContext._drain_and_barrier = _drain_and_barrier


# ---------------------------------------------------------------------------
# Kernel build
# ---------------------------------------------------------------------------
def _build_nc():
    _patch_tile_drain()
    nc = bass.Bass("TRN2")

    # ---- I/O ----
    xT = nc.dram_tensor("xT", [D, S], BF16, kind="ExternalInput")          # x[b].T
    wqT = nc.dram_tensor("wqT", [D, EC], BF16, kind="ExternalInput")       # (Wq/8).T slice
    wkT = nc.dram_tensor("wkT", [D, EC], BF16, kind="ExternalInput")
    wvT = nc.dram_tensor("wvT", [D, EC], BF16, kind="ExternalInput")
    woT = nc.dram_tensor("woT", [EC, D], BF16, kind="ExternalInput")       # Wo[:, sl].T
    bq2 = nc.dram_tensor("bq2", [128, 2], F32, kind="ExternalInput")       # per-chunk bias
    bk2 = nc.dram_tensor("bk2", [128, 2], F32, kind="ExternalInput")
    bvr = nc.dram_tensor("bvr", [128, EC], BF16, kind="ExternalInput")     # bv replicated
    mskN = nc.dram_tensor("mskN", [128, 128], F32, kind="ExternalInput")   # natural diag mask
    mskT = nc.dram_tensor("mskT", [128, 128], F32, kind="ExternalInput")   # transposed diag mask
    ident = nc.dram_tensor("ident", [128, 128], BF16, kind="ExternalInput")

    attnw = nc.dram_tensor("attnw", [HPC, S, S], BF16, kind="ExternalOutput")
    outp = nc.dram_tensor("outp", [S, D], F32, kind="ExternalOutput")

    NDC = D // 128  # 8 contraction chunks

    with tile.TileContext(nc) as tc:
        with (
            tc.tile_pool(name="persist", bufs=1) as pp,
            tc.tile_pool(name="work", bufs=2) as wp,
            tc.tile_pool(name="wtp", bufs=3) as wtp,
            tc.tile_pool(name="small", bufs=4) as sp,
            tc.tile_pool(name="ps", bufs=3, space="PSUM") as ps,
            tc.tile_pool(name="psa", bufs=2, space="PSUM") as psa,
        ):
            # ---- phase 0: load inputs ----
            xT_sb = pp.tile([128, NDC, S], BF16, tag="xT_sb")
            for dc in range(NDC):
                nc.sync.dma_start(out=xT_sb[:, dc, :], in_=xT[dc * 128:(dc + 1) * 128, :])
            wq_sb = pp.tile([128, NDC, EC], BF16, tag="wq_sb")
            wk_sb = pp.tile([128, NDC, EC], BF16, tag="wk_sb")
            wv_sb = pp.tile([128, NDC, EC], BF16, tag="wv_sb")
            for dst, src in ((wq_sb, wqT), (wk_sb, wkT), (wv_sb, wvT)):
                for dc in range(NDC):
                    nc.sync.dma_start(out=dst[:, dc, :], in_=src[dc * 128:(dc + 1) * 128, :])
            wo_sb = pp.tile([128, 2, D], BF16, tag="wo_sb")
            for c in range(2):
                nc.sync.dma_start(out=wo_sb[:, c, :], in_=woT[c * 128:(c + 1) * 128, :])
            bq_sb = pp.tile([128, 2], F32, tag="bq_sb")
            bk_sb = pp.tile([128, 2], F32, tag="bk_sb")
            bv_sb = pp.tile([128, EC], BF16, tag="bv_sb")
            mN_sb = pp.tile([128, 128], F32, tag="mN_sb")
            mT_sb = pp.tile([128, 128], F32, tag="mT_sb")
            id_sb = pp.tile([128, 128], BF16, tag="id_sb")
            nc.sync.dma_start(out=bq_sb, in_=bq2[:, :])
            nc.sync.dma_start(out=bk_sb, in_=bk2[:, :])
            nc.sync.dma_start(out=bv_sb, in_=bvr[:, :])
            nc.sync.dma_start(out=mN_sb, in_=mskN[:, :])
            nc.sync.dma_start(out=mT_sb, in_=mskT[:, :])
            nc.sync.dma_start(out=id_sb, in_=ident[:, :])

            # ---- per-head q/k tensors with 2 aug rows ----
            qaug = [pp.tile([66, S], BF16, tag=f"qaug{h}", name=f"qaug{h}") for h in range(HPC)]
            kaug = [pp.tile([66, S], BF16, tag=f"kaug{h}", name=f"kaug{h}") for h in range(HPC)]

            # ---- phase 1: projections ----
            # qT/kT: [e-chunk 128 (2 heads), s] via lhsT=w*T chunk, rhs=xT
            for (w_sb, b_sb, dest) in ((wq_sb, bq_sb, qaug), (wk_sb, bk_sb, kaug)):
                for ec in range(2):
                    for st in range(S // 512):
                        pt = ps.tile([128, 1024], F32, tag="big", name="pt_proj")
                        for dc in range(NDC):
                            nc.tensor.matmul(
                                pt[:, 0:512],
                                w_sb[:, dc, ec * 128:(ec + 1) * 128],
                                xT_sb[:, dc, st * 512:(st + 1) * 512],
                                start=(dc == 0), stop=(dc == NDC - 1),
                            )
                        stg = wp.tile([128, 512], BF16, tag="stg", name="stg_proj")
                        nc.scalar.activation(stg, pt[:, 0:512], AF.Identity,
                                             bias=b_sb[:, ec:ec + 1])
                        # partition-shift halves into per-head tensors via DMA
                        sl = slice(st * 512, (st + 1) * 512)
                        nc.sync.dma_start(out=dest[2 * ec][0:64, sl], in_=stg[0:64, :])
                        nc.sync.dma_start(out=dest[2 * ec + 1][0:64, sl], in_=stg[64:128, :])
            # v natural: [s-tile 128, 256]
            v_sb = pp.tile([128, NQT, EC], BF16, tag="v_sb")
            for st in range(NQT):
                pt = ps.tile([128, 1024], F32, tag="big", name="pt_vproj")
                for dc in range(NDC):
                    nc.tensor.matmul(
                        pt[:, 0:EC],
                        xT_sb[:, dc, st * 128:(st + 1) * 128],
                        wv_sb[:, dc, :],
                        start=(dc == 0), stop=(dc == NDC - 1),
                    )
                nc.vector.tensor_add(v_sb[:, st, :], pt[:, 0:EC], bv_sb)

            # ones rows of kaug
            for h in range(HPC):
                nc.vector.memset(kaug[h][64:66, :], 1.0)

            # attn_outT chunks [hd-dims 128 (2 heads), s]
            aoT = [pp.tile([128, S], BF16, tag=f"aoT{c}", name=f"aoT{c}") for c in range(2)]

            # ---- phase 2 per head ----
            for h in range(HPC):
                sums_z = sp.tile([128, NQT], F32, tag="sums", name=f"sums_{h}")
                recip_z = sp.tile([128, NQT], F32, tag="recip", name=f"recip_{h}")
                # --- natural side ---
                for qt in range(NQT):
                    kw = (qt + 1) * 128
                    nkh = (kw + 1023) // 1024
                    e_row = wp.tile([128, S], BF16, tag="e_row", name=f"e_{h}_{qt}")
                    acc2 = sp.tile([128, 2], F32, tag="acc2", name=f"acc_{h}_{qt}")
                    for kh in range(nkh):
                        cur = min(1024, kw - kh * 1024)
                        s_ps = ps.tile([128, 1024], F32, tag="big", name=f"sps_{h}_{qt}_{kh}")
                        for kb in range((cur + 511) // 512):
                            n = min(512, cur - kb * 512)
                            nc.tensor.matmul(
                                s_ps[:, kb * 512:kb * 512 + n],
                                qaug[h][0:64, qt * 128:(qt + 1) * 128],
                                kaug[h][0:64, kh * 1024 + kb * 512:kh * 1024 + kb * 512 + n],
                                start=True, stop=True,
                            )
                        dg = qt * 128 - kh * 1024  # diagonal subtile offset in this half
                        if 0 <= dg < 1024:
                            nc.vector.tensor_add(s_ps[:, dg:dg + 128],
                                                 s_ps[:, dg:dg + 128], mN_sb)
                        acc_ap = sums_z[:, qt:qt + 1] if nkh == 1 else acc2[:, kh:kh + 1]
                        nc.scalar.activation(
                            e_row[:, kh * 1024:kh * 1024 + cur], s_ps[:, 0:cur],
                            AF.Exp, accum_out=acc_ap,
                        )
                    if nkh == 2:
                        nc.vector.tensor_add(sums_z[:, qt:qt + 1],
                                             acc2[:, 0:1], acc2[:, 1:2])
                    nc.vector.reciprocal(recip_z[:, qt:qt + 1], sums_z[:, qt:qt + 1])
                    w_row = wp.tile([128, S], BF16, tag="w_row", name=f"w_{h}_{qt}")
                    nc.vector.tensor_scalar_mul(w_row[:, 0:kw], e_row[:, 0:kw],
                                                recip_z[:, qt:qt + 1])
                    nc.sync.dma_start(
                        out=attnw[h, qt * 128:(qt + 1) * 128, 0:kw],
                        in_=w_row[:, 0:kw],
                    )

                # --- -lnZ rows (hi + residual) into qaug[h][64:66] ---
                lnzn = sp.tile([128, NQT], F32, tag="lnzn", name=f"lnzn_{h}")
                nc.scalar.activation(lnzn, recip_z, AF.Ln)   # ln(1/Z) = -lnZ
                hi_bf = sp.tile([128, NQT], BF16, tag="hi_bf", name=f"hibf_{h}")
                nc.vector.tensor_copy(hi_bf, lnzn)
                hi_f = sp.tile([128, NQT], F32, tag="hi_f", name=f"hif_{h}")
                nc.vector.tensor_copy(hi_f, hi_bf)
                res_bf = sp.tile([128, NQT], BF16, tag="res_bf", name=f"resbf_{h}")
                nc.vector.tensor_sub(res_bf, lnzn, hi_f)
                for src, row in ((hi_bf, 64), (res_bf, 65)):
                    ptt = psa.tile([16, 128], BF16, tag="attn", name=f"ptt_{h}_{row}")
                    nc.tensor.transpose(ptt, src, id_sb)
                    stt = sp.tile([16, 128], BF16, tag="stt", name=f"stt_{h}_{row}")
                    nc.vector.tensor_copy(stt, ptt)
                    nc.sync.dma_start(out=qaug[h][row:row + 1, :], in_=stt[:, :])

                # --- transposed side + attn@v ---
                hc, hp = h // 2, h % 2
                for qh in range(2):
                    q0 = qh * 1024
                    pa = [psa.tile([128, 512], F32, tag="attn", name=f"pa_{h}_{qh}_{j}")
                          for j in range(2)]
                    ktmax = q0 // 128 + 8
                    for kt in range(ktmax):
                        qlo = max(q0, kt * 128)
                        n = q0 + 1024 - qlo
                        sT = ps.tile([128, 1024], F32, tag="big", name=f"sT_{h}_{qh}_{kt}")
                        for qb in range((n + 511) // 512):
                            nn_ = min(512, n - qb * 512)
                            nc.tensor.matmul(
                                sT[:, qb * 512:qb * 512 + nn_],
                                kaug[h][:, kt * 128:(kt + 1) * 128],
                                qaug[h][:, qlo + qb * 512:qlo + qb * 512 + nn_],
                                start=True, stop=True,
                            )
                        if kt * 128 >= q0:  # diagonal subtile at offset 0
                            nc.vector.tensor_add(sT[:, 0:128], sT[:, 0:128], mT_sb)
                        wt = wtp.tile([128, 1024], BF16, tag="wt", name=f"wt_{h}_{qh}_{kt}")
                        nc.scalar.activation(wt[:, 0:n], sT[:, 0:n], AF.Exp)
                        for j in range(2):
                            blo = q0 + 512 * j
                            if kt * 128 >= blo + 512:
                                continue
                            lo = max(blo, qlo)
                            ps_off = lo - blo
                            wt_off = lo - qlo
                            wdt = blo + 512 - lo
                            last_kt = min(ktmax - 1, (blo + 511) // 128)
                            nc.tensor.matmul(
                                pa[j][hp * 64:(hp + 1) * 64, ps_off:ps_off + wdt],
                                v_sb[:, kt, h * 64:(h + 1) * 64],
                                wt[:, wt_off:wt_off + wdt],
                                start=(kt == 0), stop=(kt == last_kt),
                                tile_position=(0, hp * 64),
                            )
                    for j in range(2):
                        nc.vector.tensor_copy(
                            aoT[hc][hp * 64:(hp + 1) * 64,
                                    q0 + 512 * j:q0 + 512 * (j + 1)],
                            pa[j][hp * 64:(hp + 1) * 64, :],
                        )

            if _DEBUG:
                aodbg = nc.dram_tensor("aodbg", [2, 128, S], BF16, kind="ExternalOutput")
                vdbg = nc.dram_tensor("vdbg", [128, NQT * EC], BF16, kind="ExternalOutput")
                for c in range(2):
                    nc.sync.dma_start(out=aodbg[c, :, :], in_=aoT[c][:, :])
                nc.sync.dma_start(out=vdbg[:, :], in_=v_sb[:, :, :])

            # ---- phase 3: out-projection partial ----
            for st in range(NQT):
                po = ps.tile([128, 1024], F32, tag="big", name=f"po_{st}")
                for eb in range(2):
                    for c in range(2):
                        nc.tensor.matmul(
                            po[:, eb * 512:(eb + 1) * 512],
                            aoT[c][:, st * 128:(st + 1) * 128],
                            wo_sb[:, c, eb * 512:(eb + 1) * 512],
                            start=(c == 0), stop=(c == 1),
                        )
                o_sb = wp.tile([128, 1024], F32, tag="o_sb", name=f"osb_{st}")
                nc.vector.tensor_copy(o_sb, po)
                nc.sync.dma_start(out=outp[st * 128:(st + 1) * 128, :], in_=o_sb)

    return nc


_NC = None


def _get_nc():
    global _NC
    if _NC is None:
        _NC = _build_nc()
    return _NC


# ---------------------------------------------------------------------------
# Host wrapper
# ---------------------------------------------------------------------------
def _prep_core_inputs(c, x, Wq, bq, Wk, bk, Wv, bv, Wo, bo, masks):
    b = c // 4
    sl = slice((c % 4) * HPC * HD, (c % 4) * HPC * HD + EC)
    bf = ml_dtypes.bfloat16
    sc = 1.0 / np.sqrt(HD)
    xT = np.ascontiguousarray(x[b].T).astype(bf)
    wqT = np.ascontiguousarray((Wq[sl] * sc).T).astype(bf)
    wkT = np.ascontiguousarray(Wk[sl].T).astype(bf)
    wvT = np.ascontiguousarray(Wv[sl].T).astype(bf)
    woT = np.ascontiguousarray(Wo[:, sl].T).astype(bf)
    bq2 = np.ascontiguousarray((bq[sl] * sc).reshape(2, 128).T).astype(np.float32)
    bk2 = np.ascontiguousarray(bk[sl].reshape(2, 128).T).astype(np.float32)
    bvr = np.broadcast_to(bv[sl], (128, EC)).astype(bf)
    mN, mT, ident = masks
    return {
        "xT": xT, "wqT": wqT, "wkT": wkT, "wvT": wvT, "woT": woT,
        "bq2": bq2, "bk2": bk2, "bvr": bvr,
        "mskN": mN, "mskT": mT, "ident": ident,
    }


def _ensure_ntff_hook():
    """Install an antenv.axon_hooks shim (missing from this image) so
    run_bass_kernel_spmd(trace=True) can capture NTFF profiles via the
    axon PJRT .so — mirrors trn_agent_boot._ntff_profile_via_ctypes."""
    import sys as _sys, types, contextlib as _ctx, ctypes
    try:
        from antenv.axon_hooks import get_axon_ntff_profile_hook  # noqa: F401
        return True
    except ImportError:
        pass
    so_path = "/opt/axon/libaxon_pjrt.so"
    if not os.path.exists(so_path):
        return False
    lib = ctypes.CDLL(so_path)
    if not hasattr(lib, "axon_start_nrt_profile"):
        return False
    lib.axon_start_nrt_profile.argtypes = [ctypes.POINTER(ctypes.c_int64), ctypes.c_size_t]
    lib.axon_start_nrt_profile.restype = ctypes.c_int64
    lib.axon_stop_nrt_profile.argtypes = [ctypes.c_char_p]
    lib.axon_stop_nrt_profile.restype = ctypes.c_int64

    @_ctx.contextmanager
    def _hook(output_dir, device_ids):
        import jax
        jax.devices()
        if device_ids:
            ids = (ctypes.c_int64 * len(device_ids))(*device_ids)
            rc = lib.axon_start_nrt_profile(ids, len(device_ids))
        else:
            rc = lib.axon_start_nrt_profile(None, 0)
        if rc != 0:
            raise RuntimeError(f"axon_start_nrt_profile rc={rc}")
        try:
            yield
        finally:
            n = lib.axon_stop_nrt_profile(str(output_dir).encode())
            print(f"ntff profile: {n} file(s) -> {output_dir}")

    import antenv
    mod = types.ModuleType("antenv.axon_hooks")
    mod.get_axon_ntff_profile_hook = lambda: _hook
    mod.set_axon_ntff_profile_hook = lambda h: None
    antenv.axon_hooks = mod
    _sys.modules["antenv.axon_hooks"] = mod
    # artifacts upload needs a bucket; keep everything local instead
    bass_utils.upload_artifacts = lambda tmpdir: tmpdir
    return True


def _run(inputs, trace=False):
    if trace:
        _ensure_ntff_hook()
    nc = _get_nc()
    x = np.asarray(inputs["x"], np.float32)
    args = (x, np.asarray(inputs["Wq"], np.float32), np.asarray(inputs["bq"], np.float32),
            np.asarray(inputs["Wk"], np.float32), np.asarray(inputs["bk"], np.float32),
            np.asarray(inputs["Wv"], np.float32), np.asarray(inputs["bv"], np.float32),
            np.asarray(inputs["Wo"], np.float32), np.asarray(inputs["bo"], np.float32))
    bo = args[8]

    ii, jj = np.meshgrid(np.arange(128), np.arange(128), indexing="ij")
    mN = np.where(jj <= ii, 0.0, MASK_VAL).astype(np.float32)
    mT = mN.T.copy()
    ident = np.eye(128, dtype=ml_dtypes.bfloat16)
    masks = (mN, mT, ident)

    in_maps = [_prep_core_inputs(c, *args, masks) for c in range(NCORES)]
    res = bass_utils.run_bass_kernel_spmd(
        nc, in_maps, core_ids=list(range(NCORES)), trace=trace,
    )

    attn = np.zeros((B, H, S, S), dtype=np.float32)
    out = np.zeros((B, S, D), dtype=np.float32)
    for c in range(NCORES):
        b = c // 4
        h0 = (c % 4) * HPC
        attn[b, h0:h0 + HPC] = np.asarray(res.results[c]["attnw"]).astype(np.float32)
        out[b] += np.asarray(res.results[c]["outp"]).astype(np.float32)
    out += bo.astype(np.float32)
    return (out, attn), res


def kernel(**inputs):
    (out, attn), _ = _run(inputs, trace=False)
    return (out, attn)


def kernel_traced(**inputs):
    """Like kernel() but also returns BassKernelResults (exec_time_ns etc.)."""
    return _run(inputs, trace=True)


# revision 11
# speedup vs baseline: 1.0931x; 1.0931x over previous
"""Causal multi-head attention (B=2, S=2048, D=1024, H=16, hd=64) for 8 Trainium2
NeuronCores, returning (output, attn_weights) like torch nn.MultiheadAttention.

Sharding: core c handles batch b=c//4 and 4 heads (c%4)*4..+4 (data+tensor
parallel per the Megatron hint). Each core computes its heads' causal attention
weights (written bf16, upper triangle left to the runtime's zero-init) and a
partial output projection; the host sums partials and adds biases.

Device-side dataflow per head:
  natural side : scores = qT.T @ kT (PE, bf16, K=64) -> +mask on diagonal tile
                 -> exp on ACT with accum_out row-sums -> w = e * (1/Z) on DVE
                 -> DMA causal rows to HBM
  transposed   : scoresT = kT_aug.T @ qT_aug with two extra contraction rows
                 carrying ones * (-lnZ) (hi + bf16 residual), so exp gives the
                 *normalized* wT directly -> attn_outT += v.T-stationary matmuls
                 -> out-projection partial from attn_outT.
"""

import numpy as np
import ml_dtypes

import concourse.bass as bass
import concourse.mybir as mybir
import concourse.tile as tile
from concourse import bass_utils
from concourse.vector_clock import ScopedClock

BF16 = mybir.dt.bfloat16
F32 = mybir.dt.float32
AF = mybir.ActivationFunctionType

import os
_DEBUG = bool(os.environ.get("KERNEL_DEBUG"))

B, S, D, H = 2, 2048, 1024, 16
HD = D // H            # 64
HPC = 4                # heads per core
EC = HPC * HD          # 256 e-dims per core
NCORES = 8
NQT = S // 128         # 16 q row-tiles
MASK_VAL = -30000.0


# ---------------------------------------------------------------------------
# Container-walrus workaround: CTRL/Drain instructions only support one
# sync-wait slot; Tile's kernel-tail drain carries one wait per DMA-HW queue.
# Split extras onto single-wait NOPs before the all-engine barrier.
# ---------------------------------------------------------------------------
_patched = False


def _patch_tile_drain():
    global _patched
    if _patched:
        return
    _patched = True

    orig_add = tile.TileContext._add_instruction

    def _add_instruction(self, inst):
        si = getattr(inst, "sync_info", None)
        eng = getattr(inst, "engine", None)
        if (
            si is not None
            and si.on_wait
            and len(si.on_wait) > 1
            and eng is not None
            and eng != mybir.EngineType.Unassigned
        ):
            waits = list(si.on_wait)
            si.on_wait = [waits[-1]]
            nc = self.nc
            for w in waits[:-1]:
                nop = mybir.InstNoOp(
                    name=f"I-waitsplit-{nc.next_id()}",
                    sync_info=mybir.SyncInfo(on_wait=[w], on_update=[]),
                    engine=eng,
                    bass_nofuse=True,
                )
                orig_add(self, nop)
        orig_add(self, inst)

    tile.TileContext._add_instruction = _add_instruction

    def _drain_and_barrier(self, tick_clock, wait_clock):
        nc = self.nc
        drain_inst = nc.sync.drain()
        wait_clock.add_sem_waits(
            drain_inst.ins, ScopedClock({None: tick_clock.global_clock})
        )
        si = drain_inst.ins.sync_info
        if si is not None and si.on_wait and len(si.on_wait) > 1:
            waits = list(si.on_wait)
            si.on_wait = [waits[0]]
            for w in waits[1:]:
                nop = mybir.InstNoOp(
                    name=f"I-waitsplit-{nc.next_id()}",
                    sync_info=mybir.SyncInfo(on_wait=[w], on_update=[]),
                    engine=drain_inst.ins.engine,
                    bass_nofuse=True,
                )
                nc.register_instruction(nop, overwrite=True)
                nc.cur_bb.bb.add_instruction(nop)

        nc.all_engine_barrier()
        assert self.sems is not None
        popped = nc._tile_sem_poison_stack.pop()
        assert popped is self._sem_poison
        nc.clear_and_free_semaphores(list(self.sems.allocated().values()))
        nc.all_engine_barrier()

    tile.TileContext._drain_and_barrier = _drain_and_barrier


# ---------------------------------------------------------------------------
# Kernel build
# ---------------------------------------------------------------------------
def _build_nc():
    _patch_tile_drain()
    nc = bass.Bass("TRN2")

    # ---- I/O ----
    xT = nc.dram_tensor("xT", [D, S], BF16, kind="ExternalInput")          # x[b].T
    wqT = nc.dram_tensor("wqT", [D, EC], BF16, kind="ExternalInput")       # (Wq/8).T slice
    wkT = nc.dram_tensor("wkT", [D, EC], BF16, kind="ExternalInput")
    wvT = nc.dram_tensor("wvT", [D, EC], BF16, kind="ExternalInput")
    woT = nc.dram_tensor("woT", [EC, D], BF16, kind="ExternalInput")       # Wo[:, sl].T
    bq2 = nc.dram_tensor("bq2", [128, 2], F32, kind="ExternalInput")       # per-chunk bias
    bk2 = nc.dram_tensor("bk2", [128, 2], F32, kind="ExternalInput")
    bvr = nc.dram_tensor("bvr", [128, EC], BF16, kind="ExternalInput")     # bv replicated
    mskN = nc.dram_tensor("mskN", [128, 128], F32, kind="ExternalInput")   # natural diag mask
    mskT = nc.dram_tensor("mskT", [128, 128], F32, kind="ExternalInput")   # transposed diag mask
    ident = nc.dram_tensor("ident", [128, 128], BF16, kind="ExternalInput")

    attnw = nc.dram_tensor("attnw", [HPC, S, S], BF16, kind="ExternalOutput")
    outp = nc.dram_tensor("outp", [S, D], F32, kind="ExternalOutput")

    NDC = D // 128  # 8 contraction chunks

    with tile.TileContext(nc) as tc:
        with (
            tc.tile_pool(name="persist", bufs=1) as pp,
            tc.tile_pool(name="work", bufs=3) as wp,
            tc.tile_pool(name="wtp", bufs=3) as wtp,
            tc.tile_pool(name="small", bufs=4) as sp,
            tc.tile_pool(name="ps", bufs=3, space="PSUM") as ps,
            tc.tile_pool(name="psa", bufs=2, space="PSUM") as psa,
        ):
            # ---- phase 0: load inputs ----
            xT_sb = pp.tile([128, NDC, S], BF16, tag="xT_sb")
            for st in range(4):
                for dc in range(NDC):
                    nc.sync.dma_start(
                        out=xT_sb[:, dc, st * 512:(st + 1) * 512],
                        in_=xT[dc * 128:(dc + 1) * 128, st * 512:(st + 1) * 512])
            wq_sb = pp.tile([128, NDC, EC], BF16, tag="wq_sb")
            wk_sb = pp.tile([128, NDC, EC], BF16, tag="wk_sb")
            wv_sb = pp.tile([128, NDC, EC], BF16, tag="wv_sb")
            for dst, src in ((wq_sb, wqT), (wk_sb, wkT), (wv_sb, wvT)):
                for dc in range(NDC):
                    nc.sync.dma_start(out=dst[:, dc, :], in_=src[dc * 128:(dc + 1) * 128, :])
            wo_sb = pp.tile([128, 2, D], BF16, tag="wo_sb")
            for c in range(2):
                nc.sync.dma_start(out=wo_sb[:, c, :], in_=woT[c * 128:(c + 1) * 128, :])
            bq_sb = pp.tile([128, 2], F32, tag="bq_sb")
            bk_sb = pp.tile([128, 2], F32, tag="bk_sb")
            bv_sb = pp.tile([128, EC], BF16, tag="bv_sb")
            mN_sb = pp.tile([128, 128], F32, tag="mN_sb")
            mT_sb = pp.tile([128, 128], F32, tag="mT_sb")
            id_sb = pp.tile([128, 128], BF16, tag="id_sb")
            nc.sync.dma_start(out=bq_sb, in_=bq2[:, :])
            nc.sync.dma_start(out=bk_sb, in_=bk2[:, :])
            nc.sync.dma_start(out=bv_sb, in_=bvr[:, :])
            nc.sync.dma_start(out=mN_sb, in_=mskN[:, :])
            nc.sync.dma_start(out=mT_sb, in_=mskT[:, :])
            nc.sync.dma_start(out=id_sb, in_=ident[:, :])

            # ---- per-head q/k tensors with 2 aug rows ----
            qaug = [pp.tile([66, S], BF16, tag=f"qaug{h}", name=f"qaug{h}") for h in range(HPC)]
            kaug = [pp.tile([66, S], BF16, tag=f"kaug{h}", name=f"kaug{h}") for h in range(HPC)]

            # ---- phase 1: projections ----
            # qT/kT: [e-chunk 128 (2 heads), s] via lhsT=w*T chunk, rhs=xT
            for (w_sb, b_sb, dest) in ((wq_sb, bq_sb, qaug), (wk_sb, bk_sb, kaug)):
                for ec in range(2):
                    for st in range(S // 512):
                        pt = ps.tile([128, 1024], F32, tag="big", name="pt_proj")
                        for dc in range(NDC):
                            nc.tensor.matmul(
                                pt[:, 0:512],
                                w_sb[:, dc, ec * 128:(ec + 1) * 128],
                                xT_sb[:, dc, st * 512:(st + 1) * 512],
                                start=(dc == 0), stop=(dc == NDC - 1),
                            )
                        stg = wp.tile([128, 512], BF16, tag="stg", name="stg_proj")
                        nc.scalar.activation(stg, pt[:, 0:512], AF.Identity,
                                             bias=b_sb[:, ec:ec + 1])
                        # partition-shift halves into per-head tensors via DMA
                        sl = slice(st * 512, (st + 1) * 512)
                        nc.sync.dma_start(out=dest[2 * ec][0:64, sl], in_=stg[0:64, :])
                        nc.sync.dma_start(out=dest[2 * ec + 1][0:64, sl], in_=stg[64:128, :])
            # v natural: [s-tile 128, 256]
            v_sb = pp.tile([128, NQT, EC], BF16, tag="v_sb")
            for st in range(NQT):
                pt = ps.tile([128, 1024], F32, tag="big", name="pt_vproj")
                for dc in range(NDC):
                    nc.tensor.matmul(
                        pt[:, 0:EC],
                        xT_sb[:, dc, st * 128:(st + 1) * 128],
                        wv_sb[:, dc, :],
                        start=(dc == 0), stop=(dc == NDC - 1),
                    )
                nc.vector.tensor_add(v_sb[:, st, :], pt[:, 0:EC], bv_sb)

            # ones rows of kaug
            for h in range(HPC):
                nc.vector.memset(kaug[h][64:66, :], 1.0)

            # attn_outT chunks [hd-dims 128 (2 heads), s]
            aoT = [pp.tile([128, S], BF16, tag=f"aoT{c}", name=f"aoT{c}") for c in range(2)]

            # ---- phase 2 per head ----
            def emit_natural(h):
                sums_z = sp.tile([128, NQT], F32, tag="sums", name=f"sums_{h}")
                recip_z = sp.tile([128, NQT], F32, tag="recip", name=f"recip_{h}")
                # --- natural side ---
                for qt in range(NQT):
                    kw = (qt + 1) * 128
                    nkh = (kw + 1023) // 1024
                    e_row = wp.tile([128, S], BF16, tag="e_row", name=f"e_{h}_{qt}")
                    acc2 = sp.tile([128, 2], F32, tag="acc2", name=f"acc_{h}_{qt}")
                    for kh in range(nkh):
                        cur = min(1024, kw - kh * 1024)
                        s_ps = ps.tile([128, 1024], F32, tag="big", name=f"sps_{h}_{qt}_{kh}")
                        for kb in range((cur + 511) // 512):
                            n = min(512, cur - kb * 512)
                            nc.tensor.matmul(
                                s_ps[:, kb * 512:kb * 512 + n],
                                qaug[h][0:64, qt * 128:(qt + 1) * 128],
                                kaug[h][0:64, kh * 1024 + kb * 512:kh * 1024 + kb * 512 + n],
                                start=True, stop=True,
                            )
                        dg = qt * 128 - kh * 1024  # diagonal subtile offset in this half
                        if 0 <= dg < 1024:
                            nc.vector.tensor_add(s_ps[:, dg:dg + 128],
                                                 s_ps[:, dg:dg + 128], mN_sb)
                        acc_ap = sums_z[:, qt:qt + 1] if nkh == 1 else acc2[:, kh:kh + 1]
                        nc.scalar.activation(
                            e_row[:, kh * 1024:kh * 1024 + cur], s_ps[:, 0:cur],
                            AF.Exp, accum_out=acc_ap,
                        )
                    if nkh == 2:
                        nc.vector.tensor_add(sums_z[:, qt:qt + 1],
                                             acc2[:, 0:1], acc2[:, 1:2])
                    nc.vector.reciprocal(recip_z[:, qt:qt + 1], sums_z[:, qt:qt + 1])
                    w_row = wp.tile([128, S], BF16, tag="w_row", name=f"w_{h}_{qt}")
                    nc.vector.tensor_scalar_mul(w_row[:, 0:kw], e_row[:, 0:kw],
                                                recip_z[:, qt:qt + 1])
                    nc.sync.dma_start(
                        out=attnw[h, qt * 128:(qt + 1) * 128, 0:kw],
                        in_=w_row[:, 0:kw],
                    )

                # --- -lnZ rows (hi + residual) into qaug[h][64:66] ---
                lnzn = sp.tile([128, NQT], F32, tag="lnzn", name=f"lnzn_{h}")
                nc.scalar.activation(lnzn, recip_z, AF.Ln)   # ln(1/Z) = -lnZ
                hi_bf = sp.tile([128, NQT], BF16, tag="hi_bf", name=f"hibf_{h}")
                nc.vector.tensor_copy(hi_bf, lnzn)
                hi_f = sp.tile([128, NQT], F32, tag="hi_f", name=f"hif_{h}")
                nc.vector.tensor_copy(hi_f, hi_bf)
                res_bf = sp.tile([128, NQT], BF16, tag="res_bf", name=f"resbf_{h}")
                nc.vector.tensor_sub(res_bf, lnzn, hi_f)
                for src, row in ((hi_bf, 64), (res_bf, 65)):
                    ptt = psa.tile([16, 128], BF16, tag="attn", name=f"ptt_{h}_{row}")
                    nc.tensor.transpose(ptt, src, id_sb)
                    stt = sp.tile([16, 128], BF16, tag="stt", name=f"stt_{h}_{row}")
                    nc.vector.tensor_copy(stt, ptt)
                    nc.sync.dma_start(out=qaug[h][row:row + 1, :], in_=stt[:, :])

            def emit_transposed(h):
                # --- transposed side + attn@v ---
                hc, hp = h // 2, h % 2
                for qh in range(2):
                    q0 = qh * 1024
                    pa = [psa.tile([128, 512], F32, tag="attn", name=f"pa_{h}_{qh}_{j}")
                          for j in range(2)]
                    ktmax = q0 // 128 + 8
                    for kt in range(ktmax):
                        qlo = max(q0, kt * 128)
                        n = q0 + 1024 - qlo
                        sT = ps.tile([128, 1024], F32, tag="big", name=f"sT_{h}_{qh}_{kt}")
                        for qb in range((n + 511) // 512):
                            nn_ = min(512, n - qb * 512)
                            nc.tensor.matmul(
                                sT[:, qb * 512:qb * 512 + nn_],
                                kaug[h][:, kt * 128:(kt + 1) * 128],
                                qaug[h][:, qlo + qb * 512:qlo + qb * 512 + nn_],
                                start=True, stop=True,
                            )
                        if kt * 128 >= q0:  # diagonal subtile at offset 0
                            nc.vector.tensor_add(sT[:, 0:128], sT[:, 0:128], mT_sb)
                        wt = wtp.tile([128, 1024], BF16, tag="wt", name=f"wt_{h}_{qh}_{kt}")
                        nc.scalar.activation(wt[:, 0:n], sT[:, 0:n], AF.Exp)
                        for j in range(2):
                            blo = q0 + 512 * j
                            if kt * 128 >= blo + 512:
                                continue
                            lo = max(blo, qlo)
                            ps_off = lo - blo
                            wt_off = lo - qlo
                            wdt = blo + 512 - lo
                            last_kt = min(ktmax - 1, (blo + 511) // 128)
                            nc.tensor.matmul(
                                pa[j][hp * 64:(hp + 1) * 64, ps_off:ps_off + wdt],
                                v_sb[:, kt, h * 64:(h + 1) * 64],
                                wt[:, wt_off:wt_off + wdt],
                                start=(kt == 0), stop=(kt == last_kt),
                                tile_position=(0, hp * 64),
                            )
                    for j in range(2):
                        nc.vector.tensor_copy(
                            aoT[hc][hp * 64:(hp + 1) * 64,
                                    q0 + 512 * j:q0 + 512 * (j + 1)],
                            pa[j][hp * 64:(hp + 1) * 64, :],
                        )

            # Interleave heads so the PE always has independent matmul work
            # while head h's lnZ chain (ACT/DVE/DMA round-trip) resolves.
            for h in range(HPC):
                emit_natural(h)
                if h >= 1:
                    emit_transposed(h - 1)
            emit_transposed(HPC - 1)

            if _DEBUG:
                aodbg = nc.dram_tensor("aodbg", [2, 128, S], BF16, kind="ExternalOutput")
                vdbg = nc.dram_tensor("vdbg", [128, NQT * EC], BF16, kind="ExternalOutput")
                for c in range(2):
                    nc.sync.dma_start(out=aodbg[c, :, :], in_=aoT[c][:, :])
                nc.sync.dma_start(out=vdbg[:, :], in_=v_sb[:, :, :])

            # ---- phase 3: out-projection partial ----
            for st in range(NQT):
                po = ps.tile([128, 1024], F32, tag="big", name=f"po_{st}")
                for eb in range(2):
                    for c in range(2):
                        nc.tensor.matmul(
                            po[:, eb * 512:(eb + 1) * 512],
                            aoT[c][:, st * 128:(st + 1) * 128],
                            wo_sb[:, c, eb * 512:(eb + 1) * 512],
                            start=(c == 0), stop=(c == 1),
                        )
                o_sb = wp.tile([128, 1024], F32, tag="o_sb", name=f"osb_{st}")
                nc.vector.tensor_copy(o_sb, po)
                nc.sync.dma_start(out=outp[st * 128:(st + 1) * 128, :], in_=o_sb)

    return nc


_NC = None


def _get_nc():
    global _NC
    if _NC is None:
        _NC = _build_nc()
    return _NC


# ---------------------------------------------------------------------------
# Host wrapper
# ---------------------------------------------------------------------------
def _prep_core_inputs(c, x, Wq, bq, Wk, bk, Wv, bv, Wo, bo, masks):
    b = c // 4
    sl = slice((c % 4) * HPC * HD, (c % 4) * HPC * HD + EC)
    bf = ml_dtypes.bfloat16
    sc = 1.0 / np.sqrt(HD)
    xT = np.ascontiguousarray(x[b].T).astype(bf)
    wqT = np.ascontiguousarray((Wq[sl] * sc).T).astype(bf)
    wkT = np.ascontiguousarray(Wk[sl].T).astype(bf)
    wvT = np.ascontiguousarray(Wv[sl].T).astype(bf)
    woT = np.ascontiguousarray(Wo[:, sl].T).astype(bf)
    bq2 = np.ascontiguousarray((bq[sl] * sc).reshape(2, 128).T).astype(np.float32)
    bk2 = np.ascontiguousarray(bk[sl].reshape(2, 128).T).astype(np.float32)
    bvr = np.broadcast_to(bv[sl], (128, EC)).astype(bf)
    mN, mT, ident = masks
    return {
        "xT": xT, "wqT": wqT, "wkT": wkT, "wvT": wvT, "woT": woT,
        "bq2": bq2, "bk2": bk2, "bvr": bvr,
        "mskN": mN, "mskT": mT, "ident": ident,
    }


def _ensure_ntff_hook():
    """Install an antenv.axon_hooks shim (missing from this image) so
    run_bass_kernel_spmd(trace=True) can capture NTFF profiles via the
    axon PJRT .so — mirrors trn_agent_boot._ntff_profile_via_ctypes."""
    import sys as _sys, types, contextlib as _ctx, ctypes
    try:
        from antenv.axon_hooks import get_axon_ntff_profile_hook  # noqa: F401
        return True
    except ImportError:
        pass
    so_path = "/opt/axon/libaxon_pjrt.so"
    if not os.path.exists(so_path):
        return False
    lib = ctypes.CDLL(so_path)
    if not hasattr(lib, "axon_start_nrt_profile"):
        return False
    lib.axon_start_nrt_profile.argtypes = [ctypes.POINTER(ctypes.c_int64), ctypes.c_size_t]
    lib.axon_start_nrt_profile.restype = ctypes.c_int64
    lib.axon_stop_nrt_profile.argtypes = [ctypes.c_char_p]
    lib.axon_stop_nrt_profile.restype = ctypes.c_int64

    @_ctx.contextmanager
    def _hook(output_dir, device_ids):
        import jax
        jax.devices()
        if device_ids:
            ids = (ctypes.c_int64 * len(device_ids))(*device_ids)
            rc = lib.axon_start_nrt_profile(ids, len(device_ids))
        else:
            rc = lib.axon_start_nrt_profile(None, 0)
        if rc != 0:
            raise RuntimeError(f"axon_start_nrt_profile rc={rc}")
        try:
            yield
        finally:
            n = lib.axon_stop_nrt_profile(str(output_dir).encode())
            print(f"ntff profile: {n} file(s) -> {output_dir}")

    import antenv
    mod = types.ModuleType("antenv.axon_hooks")
    mod.get_axon_ntff_profile_hook = lambda: _hook
    mod.set_axon_ntff_profile_hook = lambda h: None
    antenv.axon_hooks = mod
    _sys.modules["antenv.axon_hooks"] = mod
    # artifacts upload needs a bucket; keep everything local instead
    bass_utils.upload_artifacts = lambda tmpdir: tmpdir
    return True


def _run(inputs, trace=False):
    if trace:
        _ensure_ntff_hook()
    nc = _get_nc()
    x = np.asarray(inputs["x"], np.float32)
    args = (x, np.asarray(inputs["Wq"], np.float32), np.asarray(inputs["bq"], np.float32),
            np.asarray(inputs["Wk"], np.float32), np.asarray(inputs["bk"], np.float32),
            np.asarray(inputs["Wv"], np.float32), np.asarray(inputs["bv"], np.float32),
            np.asarray(inputs["Wo"], np.float32), np.asarray(inputs["bo"], np.float32))
    bo = args[8]

    ii, jj = np.meshgrid(np.arange(128), np.arange(128), indexing="ij")
    mN = np.where(jj <= ii, 0.0, MASK_VAL).astype(np.float32)
    mT = mN.T.copy()
    ident = np.eye(128, dtype=ml_dtypes.bfloat16)
    masks = (mN, mT, ident)

    in_maps = [_prep_core_inputs(c, *args, masks) for c in range(NCORES)]
    res = bass_utils.run_bass_kernel_spmd(
        nc, in_maps, core_ids=list(range(NCORES)), trace=trace,
    )

    attn = np.zeros((B, H, S, S), dtype=np.float32)
    out = np.zeros((B, S, D), dtype=np.float32)
    for c in range(NCORES):
        b = c // 4
        h0 = (c % 4) * HPC
        attn[b, h0:h0 + HPC] = np.asarray(res.results[c]["attnw"]).astype(np.float32)
        out[b] += np.asarray(res.results[c]["outp"]).astype(np.float32)
    out += bo.astype(np.float32)
    return (out, attn), res


def kernel(**inputs):
    (out, attn), _ = _run(inputs, trace=False)
    return (out, attn)


def kernel_traced(**inputs):
    """Like kernel() but also returns BassKernelResults (exec_time_ns etc.)."""
    return _run(inputs, trace=True)


# revision 17
# speedup vs baseline: 1.1007x; 1.0069x over previous
"""Causal multi-head attention (B=2, S=2048, D=1024, H=16, hd=64) for 8 Trainium2
NeuronCores, returning (output, attn_weights) like torch nn.MultiheadAttention.

Sharding: core c handles batch b=c//4 and 4 heads (c%4)*4..+4 (data+tensor
parallel per the Megatron hint). Each core computes its heads' causal attention
weights (written bf16, upper triangle left to the runtime's zero-init) and a
partial output projection; the host sums partials and adds biases.

Device-side dataflow per head:
  natural side : scores = qT.T @ kT (PE, bf16, K=64) -> +mask on diagonal tile
                 -> exp on ACT with accum_out row-sums -> w = e * (1/Z) on DVE
                 -> DMA causal rows to HBM
  transposed   : scoresT = kT_aug.T @ qT_aug with two extra contraction rows
                 carrying ones * (-lnZ) (hi + bf16 residual), so exp gives the
                 *normalized* wT directly -> attn_outT += v.T-stationary matmuls
                 -> out-projection partial from attn_outT.
"""

import numpy as np
import ml_dtypes

import concourse.bass as bass
import concourse.mybir as mybir
import concourse.tile as tile
from concourse import bass_utils
from concourse.vector_clock import ScopedClock

BF16 = mybir.dt.bfloat16
F32 = mybir.dt.float32
AF = mybir.ActivationFunctionType

import os
_DEBUG = bool(os.environ.get("KERNEL_DEBUG"))

B, S, D, H = 2, 2048, 1024, 16
HD = D // H            # 64
HPC = 4                # heads per core
EC = HPC * HD          # 256 e-dims per core
NCORES = 8
NQT = S // 128         # 16 q row-tiles
MASK_VAL = -30000.0


# ---------------------------------------------------------------------------
# Container-walrus workaround: CTRL/Drain instructions only support one
# sync-wait slot; Tile's kernel-tail drain carries one wait per DMA-HW queue.
# Split extras onto single-wait NOPs before the all-engine barrier.
# ---------------------------------------------------------------------------
_patched = False


def _patch_tile_drain():
    global _patched
    if _patched:
        return
    _patched = True

    orig_add = tile.TileContext._add_instruction

    def _add_instruction(self, inst):
        si = getattr(inst, "sync_info", None)
        eng = getattr(inst, "engine", None)
        if (
            si is not None
            and si.on_wait
            and len(si.on_wait) > 1
            and eng is not None
            and eng != mybir.EngineType.Unassigned
        ):
            waits = list(si.on_wait)
            si.on_wait = [waits[-1]]
            nc = self.nc
            for w in waits[:-1]:
                nop = mybir.InstNoOp(
                    name=f"I-waitsplit-{nc.next_id()}",
                    sync_info=mybir.SyncInfo(on_wait=[w], on_update=[]),
                    engine=eng,
                    bass_nofuse=True,
                )
                orig_add(self, nop)
        orig_add(self, inst)

    tile.TileContext._add_instruction = _add_instruction

    def _drain_and_barrier(self, tick_clock, wait_clock):
        nc = self.nc
        drain_inst = nc.sync.drain()
        wait_clock.add_sem_waits(
            drain_inst.ins, ScopedClock({None: tick_clock.global_clock})
        )
        si = drain_inst.ins.sync_info
        if si is not None and si.on_wait and len(si.on_wait) > 1:
            waits = list(si.on_wait)
            si.on_wait = [waits[0]]
            for w in waits[1:]:
                nop = mybir.InstNoOp(
                    name=f"I-waitsplit-{nc.next_id()}",
                    sync_info=mybir.SyncInfo(on_wait=[w], on_update=[]),
                    engine=drain_inst.ins.engine,
                    bass_nofuse=True,
                )
                nc.register_instruction(nop, overwrite=True)
                nc.cur_bb.bb.add_instruction(nop)

        nc.all_engine_barrier()
        assert self.sems is not None
        popped = nc._tile_sem_poison_stack.pop()
        assert popped is self._sem_poison
        nc.clear_and_free_semaphores(list(self.sems.allocated().values()))
        nc.all_engine_barrier()

    tile.TileContext._drain_and_barrier = _drain_and_barrier


# ---------------------------------------------------------------------------
# Kernel build
# ---------------------------------------------------------------------------
def _build_nc():
    _patch_tile_drain()
    nc = bass.Bass("TRN2")

    # ---- I/O ----
    xT = nc.dram_tensor("xT", [D, S], BF16, kind="ExternalInput")          # x[b].T
    wqT = nc.dram_tensor("wqT", [D, EC], BF16, kind="ExternalInput")       # (Wq/8).T slice
    wkT = nc.dram_tensor("wkT", [D, EC], BF16, kind="ExternalInput")
    wvT = nc.dram_tensor("wvT", [D, EC], BF16, kind="ExternalInput")
    woT = nc.dram_tensor("woT", [EC, D], BF16, kind="ExternalInput")       # Wo[:, sl].T
    bq2 = nc.dram_tensor("bq2", [128, 2], F32, kind="ExternalInput")       # per-chunk bias
    bk2 = nc.dram_tensor("bk2", [128, 2], F32, kind="ExternalInput")
    bvr = nc.dram_tensor("bvr", [128, EC], BF16, kind="ExternalInput")     # bv replicated
    mskN = nc.dram_tensor("mskN", [128, 128], F32, kind="ExternalInput")   # natural diag mask
    mskT = nc.dram_tensor("mskT", [128, 128], F32, kind="ExternalInput")   # transposed diag mask
    ident = nc.dram_tensor("ident", [128, 128], BF16, kind="ExternalInput")

    attnw = nc.dram_tensor("attnw", [HPC, S, S], BF16, kind="ExternalOutput")
    outp = nc.dram_tensor("outp", [S, D], F32, kind="ExternalOutput")

    NDC = D // 128  # 8 contraction chunks

    with tile.TileContext(nc) as tc:
        with (
            tc.tile_pool(name="persist", bufs=1) as pp,
            tc.tile_pool(name="work", bufs=3) as wp,
            tc.tile_pool(name="wtp", bufs=3) as wtp,
            tc.tile_pool(name="small", bufs=4) as sp,
            tc.tile_pool(name="ps", bufs=6, space="PSUM") as ps,
            tc.tile_pool(name="psa", bufs=2, space="PSUM") as psa,
        ):
            # ---- phase 0: load inputs ----
            xT_sb = pp.tile([128, NDC, S], BF16, tag="xT_sb")
            for st in range(4):
                for dc in range(NDC):
                    nc.sync.dma_start(
                        out=xT_sb[:, dc, st * 512:(st + 1) * 512],
                        in_=xT[dc * 128:(dc + 1) * 128, st * 512:(st + 1) * 512])
            wq_sb = pp.tile([128, NDC, EC], BF16, tag="wq_sb")
            wk_sb = pp.tile([128, NDC, EC], BF16, tag="wk_sb")
            wv_sb = pp.tile([128, NDC, EC], BF16, tag="wv_sb")
            for dst, src in ((wq_sb, wqT), (wk_sb, wkT), (wv_sb, wvT)):
                for dc in range(NDC):
                    nc.sync.dma_start(out=dst[:, dc, :], in_=src[dc * 128:(dc + 1) * 128, :])
            wo_sb = pp.tile([128, 2, D], BF16, tag="wo_sb")
            for c in range(2):
                nc.sync.dma_start(out=wo_sb[:, c, :], in_=woT[c * 128:(c + 1) * 128, :])
            bq_sb = pp.tile([128, 2], F32, tag="bq_sb")
            bk_sb = pp.tile([128, 2], F32, tag="bk_sb")
            bv_sb = pp.tile([128, EC], BF16, tag="bv_sb")
            mN_sb = pp.tile([128, 128], F32, tag="mN_sb")
            mT_sb = pp.tile([128, 128], F32, tag="mT_sb")
            id_sb = pp.tile([128, 128], BF16, tag="id_sb")
            nc.sync.dma_start(out=bq_sb, in_=bq2[:, :])
            nc.sync.dma_start(out=bk_sb, in_=bk2[:, :])
            nc.sync.dma_start(out=bv_sb, in_=bvr[:, :])
            nc.sync.dma_start(out=mN_sb, in_=mskN[:, :])
            nc.sync.dma_start(out=mT_sb, in_=mskT[:, :])
            nc.sync.dma_start(out=id_sb, in_=ident[:, :])

            # ---- per-head q/k tensors with 2 aug rows ----
            qaug = [pp.tile([66, S], BF16, tag=f"qaug{h}", name=f"qaug{h}") for h in range(HPC)]
            kaug = [pp.tile([66, S], BF16, tag=f"kaug{h}", name=f"kaug{h}") for h in range(HPC)]

            # ---- phase 1: projections ----
            # qT/kT: [e-chunk 128 (2 heads), s] via lhsT=w*T chunk, rhs=xT
            for (w_sb, b_sb, dest) in ((wq_sb, bq_sb, qaug), (wk_sb, bk_sb, kaug)):
                for ec in range(2):
                    for st in range(S // 512):
                        pt = ps.tile([128, 512], F32, tag="big", name="pt_proj")
                        for dc in range(NDC):
                            nc.tensor.matmul(
                                pt[:, :],
                                w_sb[:, dc, ec * 128:(ec + 1) * 128],
                                xT_sb[:, dc, st * 512:(st + 1) * 512],
                                start=(dc == 0), stop=(dc == NDC - 1),
                            )
                        stg = wp.tile([128, 512], BF16, tag="stg", name="stg_proj")
                        nc.scalar.activation(stg, pt[:, :], AF.Identity,
                                             bias=b_sb[:, ec:ec + 1])
                        # partition-shift halves into per-head tensors via DMA
                        sl = slice(st * 512, (st + 1) * 512)
                        nc.sync.dma_start(out=dest[2 * ec][0:64, sl], in_=stg[0:64, :])
                        nc.sync.dma_start(out=dest[2 * ec + 1][0:64, sl], in_=stg[64:128, :])
            # v natural: [s-tile 128, 256]
            v_sb = pp.tile([128, NQT, EC], BF16, tag="v_sb")
            for st in range(NQT):
                pt = ps.tile([128, 512], F32, tag="big", name="pt_vproj")
                for dc in range(NDC):
                    nc.tensor.matmul(
                        pt[:, 0:EC],
                        xT_sb[:, dc, st * 128:(st + 1) * 128],
                        wv_sb[:, dc, :],
                        start=(dc == 0), stop=(dc == NDC - 1),
                    )
                nc.vector.tensor_add(v_sb[:, st, :], pt[:, 0:EC], bv_sb)

            # ones rows of kaug
            for h in range(HPC):
                nc.vector.memset(kaug[h][64:66, :], 1.0)

            # attn_outT chunks [hd-dims 128 (2 heads), s]
            aoT = [pp.tile([128, S], BF16, tag=f"aoT{c}", name=f"aoT{c}") for c in range(2)]

            # ---- phase 2 per head ----
            def emit_natural(h):
                sums_z = sp.tile([128, NQT], F32, tag="sums", name=f"sums_{h}")
                recip_z = sp.tile([128, NQT], F32, tag="recip", name=f"recip_{h}")
                # --- natural side ---
                for qt in range(NQT):
                    kw = (qt + 1) * 128
                    nkb = (kw + 511) // 512
                    e_row = wp.tile([128, S], BF16, tag="e_row", name=f"e_{h}_{qt}")
                    acc4 = sp.tile([128, 4], F32, tag="acc4", name=f"acc_{h}_{qt}")
                    for kb in range(nkb):
                        n = min(512, kw - kb * 512)
                        s_ps = ps.tile([128, 512], F32, tag="big", name=f"sps_{h}_{qt}_{kb}")
                        nc.tensor.matmul(
                            s_ps[:, 0:n],
                            qaug[h][0:64, qt * 128:(qt + 1) * 128],
                            kaug[h][0:64, kb * 512:kb * 512 + n],
                            start=True, stop=True,
                        )
                        dg = qt * 128 - kb * 512  # diagonal subtile offset in block
                        if 0 <= dg < 512:
                            nc.vector.tensor_add(s_ps[:, dg:dg + 128],
                                                 s_ps[:, dg:dg + 128], mN_sb)
                        acc_ap = sums_z[:, qt:qt + 1] if nkb == 1 else acc4[:, kb:kb + 1]
                        nc.scalar.activation(
                            e_row[:, kb * 512:kb * 512 + n], s_ps[:, 0:n],
                            AF.Exp, accum_out=acc_ap,
                        )
                    if nkb > 1:
                        nc.vector.tensor_reduce(
                            sums_z[:, qt:qt + 1], acc4[:, 0:nkb],
                            axis=mybir.AxisListType.X, op=mybir.AluOpType.add,
                        )
                    nc.vector.reciprocal(recip_z[:, qt:qt + 1], sums_z[:, qt:qt + 1])
                    w_row = wp.tile([128, S], BF16, tag="w_row", name=f"w_{h}_{qt}")
                    nc.vector.tensor_scalar_mul(w_row[:, 0:kw], e_row[:, 0:kw],
                                                recip_z[:, qt:qt + 1])
                    nc.sync.dma_start(
                        out=attnw[h, qt * 128:(qt + 1) * 128, 0:kw],
                        in_=w_row[:, 0:kw],
                    )
                    yield

                # --- -lnZ rows (hi + residual) into qaug[h][64:66] ---
                lnzn = sp.tile([128, NQT], F32, tag="lnzn", name=f"lnzn_{h}")
                nc.scalar.activation(lnzn, recip_z, AF.Ln)   # ln(1/Z) = -lnZ
                hi_bf = sp.tile([128, NQT], BF16, tag="hi_bf", name=f"hibf_{h}")
                nc.vector.tensor_copy(hi_bf, lnzn)
                hi_f = sp.tile([128, NQT], F32, tag="hi_f", name=f"hif_{h}")
                nc.vector.tensor_copy(hi_f, hi_bf)
                res_bf = sp.tile([128, NQT], BF16, tag="res_bf", name=f"resbf_{h}")
                nc.vector.tensor_sub(res_bf, lnzn, hi_f)
                for src, row in ((hi_bf, 64), (res_bf, 65)):
                    ptt = psa.tile([16, 128], BF16, tag="attn", name=f"ptt_{h}_{row}")
                    nc.tensor.transpose(ptt, src, id_sb)
                    stt = sp.tile([16, 128], BF16, tag="stt", name=f"stt_{h}_{row}")
                    nc.vector.tensor_copy(stt, ptt)
                    nc.sync.dma_start(out=qaug[h][row:row + 1, :], in_=stt[:, :])
                yield

            def emit_transposed(h):
                # --- transposed side + attn@v (512-grid-aligned blocks) ---
                hc, hp = h // 2, h % 2
                for qh in range(2):
                    q0 = qh * 1024
                    pa = [psa.tile([128, 512], F32, tag="attn", name=f"pa_{h}_{qh}_{j}")
                          for j in range(2)]
                    ktmax = q0 // 128 + 8
                    for kt in range(ktmax):
                        for qb in range(2):
                            blo = q0 + 512 * qb
                            lo = max(blo, kt * 128)
                            wdt = blo + 512 - lo
                            if wdt <= 0:
                                continue
                            ps_off = lo - blo
                            sT = ps.tile([128, 512], F32, tag="big",
                                         name=f"sT_{h}_{qh}_{kt}_{qb}")
                            nc.tensor.matmul(
                                sT[:, 0:wdt],
                                kaug[h][:, kt * 128:(kt + 1) * 128],
                                qaug[h][:, lo:lo + wdt],
                                start=True, stop=True,
                            )
                            if kt * 128 >= blo:  # diagonal subtile at offset 0
                                nc.vector.tensor_add(sT[:, 0:128], sT[:, 0:128], mT_sb)
                            wt = wtp.tile([128, 512], BF16, tag="wt",
                                          name=f"wt_{h}_{qh}_{kt}_{qb}")
                            nc.scalar.activation(wt[:, 0:wdt], sT[:, 0:wdt], AF.Exp)
                            last_kt = min(ktmax - 1, (blo + 511) // 128)
                            nc.tensor.matmul(
                                pa[qb][hp * 64:(hp + 1) * 64, ps_off:ps_off + wdt],
                                v_sb[:, kt, h * 64:(h + 1) * 64],
                                wt[:, 0:wdt],
                                start=(kt == 0), stop=(kt == last_kt),
                                tile_position=(0, hp * 64),
                            )
                        yield
                    for j in range(2):
                        nc.vector.tensor_copy(
                            aoT[hc][hp * 64:(hp + 1) * 64,
                                    q0 + 512 * j:q0 + 512 * (j + 1)],
                            pa[j][hp * 64:(hp + 1) * 64, :],
                        )

            # Fine-grained interleave: natural(h) zipped with transposed(h-1)
            # so the PE stream always has independent matmuls to fill the
            # bubbles left by the ACT-gated softmax pipelines (keeps HAM warm).
            def _drain(g):
                for _ in g:
                    pass

            prev = None
            for h in range(HPC):
                cur = emit_natural(h)
                if prev is None:
                    _drain(cur)
                else:
                    gens = [cur, prev]
                    while gens:
                        for g in list(gens):
                            try:
                                next(g)
                            except StopIteration:
                                gens.remove(g)
                prev = emit_transposed(h)
            _drain(prev)

            if _DEBUG:
                aodbg = nc.dram_tensor("aodbg", [2, 128, S], BF16, kind="ExternalOutput")
                vdbg = nc.dram_tensor("vdbg", [128, NQT * EC], BF16, kind="ExternalOutput")
                for c in range(2):
                    nc.sync.dma_start(out=aodbg[c, :, :], in_=aoT[c][:, :])
                nc.sync.dma_start(out=vdbg[:, :], in_=v_sb[:, :, :])

            # ---- phase 3: out-projection partial ----
            for st in range(NQT):
                o_sb = wp.tile([128, 1024], F32, tag="o_sb", name=f"osb_{st}")
                for eb in range(2):
                    po = ps.tile([128, 512], F32, tag="big", name=f"po_{st}_{eb}")
                    for c in range(2):
                        nc.tensor.matmul(
                            po[:, :],
                            aoT[c][:, st * 128:(st + 1) * 128],
                            wo_sb[:, c, eb * 512:(eb + 1) * 512],
                            start=(c == 0), stop=(c == 1),
                        )
                    nc.vector.tensor_copy(o_sb[:, eb * 512:(eb + 1) * 512], po)
                nc.sync.dma_start(out=outp[st * 128:(st + 1) * 128, :], in_=o_sb)

    return nc


_NC = None


def _get_nc():
    global _NC
    if _NC is None:
        _NC = _build_nc()
    return _NC


# ---------------------------------------------------------------------------
# Host wrapper
# ---------------------------------------------------------------------------
def _prep_core_inputs(c, x, Wq, bq, Wk, bk, Wv, bv, Wo, bo, masks):
    b = c // 4
    sl = slice((c % 4) * HPC * HD, (c % 4) * HPC * HD + EC)
    bf = ml_dtypes.bfloat16
    sc = 1.0 / np.sqrt(HD)
    xT = np.ascontiguousarray(x[b].T).astype(bf)
    wqT = np.ascontiguousarray((Wq[sl] * sc).T).astype(bf)
    wkT = np.ascontiguousarray(Wk[sl].T).astype(bf)
    wvT = np.ascontiguousarray(Wv[sl].T).astype(bf)
    woT = np.ascontiguousarray(Wo[:, sl].T).astype(bf)
    bq2 = np.ascontiguousarray((bq[sl] * sc).reshape(2, 128).T).astype(np.float32)
    bk2 = np.ascontiguousarray(bk[sl].reshape(2, 128).T).astype(np.float32)
    bvr = np.broadcast_to(bv[sl], (128, EC)).astype(bf)
    mN, mT, ident = masks
    return {
        "xT": xT, "wqT": wqT, "wkT": wkT, "wvT": wvT, "woT": woT,
        "bq2": bq2, "bk2": bk2, "bvr": bvr,
        "mskN": mN, "mskT": mT, "ident": ident,
    }


def _ensure_ntff_hook():
    """Install an antenv.axon_hooks shim (missing from this image) so
    run_bass_kernel_spmd(trace=True) can capture NTFF profiles via the
    axon PJRT .so — mirrors trn_agent_boot._ntff_profile_via_ctypes."""
    import sys as _sys, types, contextlib as _ctx, ctypes
    try:
        from antenv.axon_hooks import get_axon_ntff_profile_hook  # noqa: F401
        return True
    except ImportError:
        pass
    so_path = "/opt/axon/libaxon_pjrt.so"
    if not os.path.exists(so_path):
        return False
    lib = ctypes.CDLL(so_path)
    if not hasattr(lib, "axon_start_nrt_profile"):
        return False
    lib.axon_start_nrt_profile.argtypes = [ctypes.POINTER(ctypes.c_int64), ctypes.c_size_t]
    lib.axon_start_nrt_profile.restype = ctypes.c_int64
    lib.axon_stop_nrt_profile.argtypes = [ctypes.c_char_p]
    lib.axon_stop_nrt_profile.restype = ctypes.c_int64

    @_ctx.contextmanager
    def _hook(output_dir, device_ids):
        import jax
        jax.devices()
        if device_ids:
            ids = (ctypes.c_int64 * len(device_ids))(*device_ids)
            rc = lib.axon_start_nrt_profile(ids, len(device_ids))
        else:
            rc = lib.axon_start_nrt_profile(None, 0)
        if rc != 0:
            raise RuntimeError(f"axon_start_nrt_profile rc={rc}")
        try:
            yield
        finally:
            n = lib.axon_stop_nrt_profile(str(output_dir).encode())
            print(f"ntff profile: {n} file(s) -> {output_dir}")

    import antenv
    mod = types.ModuleType("antenv.axon_hooks")
    mod.get_axon_ntff_profile_hook = lambda: _hook
    mod.set_axon_ntff_profile_hook = lambda h: None
    antenv.axon_hooks = mod
    _sys.modules["antenv.axon_hooks"] = mod
    # artifacts upload needs a bucket; keep everything local instead
    bass_utils.upload_artifacts = lambda tmpdir: tmpdir
    return True


def _run(inputs, trace=False):
    if trace:
        _ensure_ntff_hook()
    nc = _get_nc()
    x = np.asarray(inputs["x"], np.float32)
    args = (x, np.asarray(inputs["Wq"], np.float32), np.asarray(inputs["bq"], np.float32),
            np.asarray(inputs["Wk"], np.float32), np.asarray(inputs["bk"], np.float32),
            np.asarray(inputs["Wv"], np.float32), np.asarray(inputs["bv"], np.float32),
            np.asarray(inputs["Wo"], np.float32), np.asarray(inputs["bo"], np.float32))
    bo = args[8]

    ii, jj = np.meshgrid(np.arange(128), np.arange(128), indexing="ij")
    mN = np.where(jj <= ii, 0.0, MASK_VAL).astype(np.float32)
    mT = mN.T.copy()
    ident = np.eye(128, dtype=ml_dtypes.bfloat16)
    masks = (mN, mT, ident)

    in_maps = [_prep_core_inputs(c, *args, masks) for c in range(NCORES)]
    res = bass_utils.run_bass_kernel_spmd(
        nc, in_maps, core_ids=list(range(NCORES)), trace=trace,
    )

    attn = np.zeros((B, H, S, S), dtype=np.float32)
    out = np.zeros((B, S, D), dtype=np.float32)
    for c in range(NCORES):
        b = c // 4
        h0 = (c % 4) * HPC
        attn[b, h0:h0 + HPC] = np.asarray(res.results[c]["attnw"]).astype(np.float32)
        out[b] += np.asarray(res.results[c]["outp"]).astype(np.float32)
    out += bo.astype(np.float32)
    return (out, attn), res


def kernel(**inputs):
    (out, attn), _ = _run(inputs, trace=False)
    return (out, attn)


def kernel_traced(**inputs):
    """Like kernel() but also returns BassKernelResults (exec_time_ns etc.)."""
    return _run(inputs, trace=True)


# revision 21
# speedup vs baseline: 1.1129x; 1.0111x over previous
"""Causal multi-head attention (B=2, S=2048, D=1024, H=16, hd=64) for 8 Trainium2
NeuronCores, returning (output, attn_weights) like torch nn.MultiheadAttention.

Sharding: core c handles batch b=c//4 and 4 heads (c%4)*4..+4 (data+tensor
parallel per the Megatron hint). Each core computes its heads' causal attention
weights (written bf16, upper triangle left to the runtime's zero-init) and a
partial output projection; the host sums partials and adds biases.

Device-side dataflow per head:
  natural side : scores = qT.T @ kT (PE, bf16, K=64) -> +mask on diagonal tile
                 -> exp on ACT with accum_out row-sums -> w = e * (1/Z) on DVE
                 -> DMA causal rows to HBM
  transposed   : scoresT = kT_aug.T @ qT_aug with two extra contraction rows
                 carrying ones * (-lnZ) (hi + bf16 residual), so exp gives the
                 *normalized* wT directly -> attn_outT += v.T-stationary matmuls
                 -> out-projection partial from attn_outT.
"""

import numpy as np
import ml_dtypes

import concourse.bass as bass
import concourse.mybir as mybir
import concourse.tile as tile
from concourse import bass_utils
from concourse.vector_clock import ScopedClock

BF16 = mybir.dt.bfloat16
F32 = mybir.dt.float32
AF = mybir.ActivationFunctionType

import os
_DEBUG = bool(os.environ.get("KERNEL_DEBUG"))

B, S, D, H = 2, 2048, 1024, 16
HD = D // H            # 64
HPC = 4                # heads per core
EC = HPC * HD          # 256 e-dims per core
NCORES = 8
NQT = S // 128         # 16 q row-tiles
MASK_VAL = -30000.0


# ---------------------------------------------------------------------------
# Container-walrus workaround: CTRL/Drain instructions only support one
# sync-wait slot; Tile's kernel-tail drain carries one wait per DMA-HW queue.
# Split extras onto single-wait NOPs before the all-engine barrier.
# ---------------------------------------------------------------------------
_patched = False


def _patch_tile_drain():
    global _patched
    if _patched:
        return
    _patched = True

    orig_add = tile.TileContext._add_instruction

    def _add_instruction(self, inst):
        si = getattr(inst, "sync_info", None)
        eng = getattr(inst, "engine", None)
        if (
            si is not None
            and si.on_wait
            and len(si.on_wait) > 1
            and eng is not None
            and eng != mybir.EngineType.Unassigned
        ):
            waits = list(si.on_wait)
            si.on_wait = [waits[-1]]
            nc = self.nc
            for w in waits[:-1]:
                nop = mybir.InstNoOp(
                    name=f"I-waitsplit-{nc.next_id()}",
                    sync_info=mybir.SyncInfo(on_wait=[w], on_update=[]),
                    engine=eng,
                    bass_nofuse=True,
                )
                orig_add(self, nop)
        orig_add(self, inst)

    tile.TileContext._add_instruction = _add_instruction

    def _drain_and_barrier(self, tick_clock, wait_clock):
        nc = self.nc
        drain_inst = nc.sync.drain()
        wait_clock.add_sem_waits(
            drain_inst.ins, ScopedClock({None: tick_clock.global_clock})
        )
        si = drain_inst.ins.sync_info
        if si is not None and si.on_wait and len(si.on_wait) > 1:
            waits = list(si.on_wait)
            si.on_wait = [waits[0]]
            for w in waits[1:]:
                nop = mybir.InstNoOp(
                    name=f"I-waitsplit-{nc.next_id()}",
                    sync_info=mybir.SyncInfo(on_wait=[w], on_update=[]),
                    engine=drain_inst.ins.engine,
                    bass_nofuse=True,
                )
                nc.register_instruction(nop, overwrite=True)
                nc.cur_bb.bb.add_instruction(nop)

        nc.all_engine_barrier()
        assert self.sems is not None
        popped = nc._tile_sem_poison_stack.pop()
        assert popped is self._sem_poison
        nc.clear_and_free_semaphores(list(self.sems.allocated().values()))
        nc.all_engine_barrier()

    tile.TileContext._drain_and_barrier = _drain_and_barrier


# ---------------------------------------------------------------------------
# Kernel build
# ---------------------------------------------------------------------------
def _build_nc():
    _patch_tile_drain()
    nc = bass.Bass("TRN2")

    # ---- I/O ----
    xT = nc.dram_tensor("xT", [D, S], BF16, kind="ExternalInput")          # x[b].T
    wqT = nc.dram_tensor("wqT", [D, EC], BF16, kind="ExternalInput")       # (Wq/8).T slice
    wkT = nc.dram_tensor("wkT", [D, EC], BF16, kind="ExternalInput")
    wvT = nc.dram_tensor("wvT", [D, EC], BF16, kind="ExternalInput")
    woT = nc.dram_tensor("woT", [EC, D], BF16, kind="ExternalInput")       # Wo[:, sl].T
    bq2 = nc.dram_tensor("bq2", [128, 2], F32, kind="ExternalInput")       # per-chunk bias
    bk2 = nc.dram_tensor("bk2", [128, 2], F32, kind="ExternalInput")
    bvr = nc.dram_tensor("bvr", [128, EC], BF16, kind="ExternalInput")     # bv replicated
    mskN = nc.dram_tensor("mskN", [128, 128], F32, kind="ExternalInput")   # natural diag mask
    mskT = nc.dram_tensor("mskT", [128, 128], F32, kind="ExternalInput")   # transposed diag mask
    ident = nc.dram_tensor("ident", [128, 128], BF16, kind="ExternalInput")

    attnw = nc.dram_tensor("attnw", [HPC, S, S], BF16, kind="ExternalOutput")
    outp = nc.dram_tensor("outp", [S, D], F32, kind="ExternalOutput")

    NDC = D // 128  # 8 contraction chunks

    with tile.TileContext(nc) as tc:
        with (
            tc.tile_pool(name="persist", bufs=1) as pp,
            tc.tile_pool(name="work", bufs=3) as wp,
            tc.tile_pool(name="wtp", bufs=3) as wtp,
            tc.tile_pool(name="small", bufs=4) as sp,
            tc.tile_pool(name="ps", bufs=3, space="PSUM") as ps,
            tc.tile_pool(name="psa", bufs=2, space="PSUM") as psa,
        ):
            # ---- phase 0: load inputs ----
            xT_sb = pp.tile([128, NDC, S], BF16, tag="xT_sb")
            for st in range(4):
                for dc in range(NDC):
                    nc.sync.dma_start(
                        out=xT_sb[:, dc, st * 512:(st + 1) * 512],
                        in_=xT[dc * 128:(dc + 1) * 128, st * 512:(st + 1) * 512])
            wq_sb = pp.tile([128, NDC, EC], BF16, tag="wq_sb")
            wk_sb = pp.tile([128, NDC, EC], BF16, tag="wk_sb")
            wv_sb = pp.tile([128, NDC, EC], BF16, tag="wv_sb")
            for dst, src in ((wq_sb, wqT), (wk_sb, wkT), (wv_sb, wvT)):
                for dc in range(NDC):
                    nc.sync.dma_start(out=dst[:, dc, :], in_=src[dc * 128:(dc + 1) * 128, :])
            wo_sb = pp.tile([128, 2, D], BF16, tag="wo_sb")
            for c in range(2):
                nc.sync.dma_start(out=wo_sb[:, c, :], in_=woT[c * 128:(c + 1) * 128, :])
            bq_sb = pp.tile([128, 2], F32, tag="bq_sb")
            bk_sb = pp.tile([128, 2], F32, tag="bk_sb")
            bv_sb = pp.tile([128, EC], BF16, tag="bv_sb")
            mN_sb = pp.tile([128, 128], F32, tag="mN_sb")
            mT_sb = pp.tile([128, 128], F32, tag="mT_sb")
            id_sb = pp.tile([128, 128], BF16, tag="id_sb")
            nc.sync.dma_start(out=bq_sb, in_=bq2[:, :])
            nc.sync.dma_start(out=bk_sb, in_=bk2[:, :])
            nc.sync.dma_start(out=bv_sb, in_=bvr[:, :])
            nc.sync.dma_start(out=mN_sb, in_=mskN[:, :])
            nc.sync.dma_start(out=mT_sb, in_=mskT[:, :])
            nc.sync.dma_start(out=id_sb, in_=ident[:, :])

            # ---- per-head q/k tensors with 2 aug rows ----
            qaug = [pp.tile([66, S], BF16, tag=f"qaug{h}", name=f"qaug{h}") for h in range(HPC)]
            kaug = [pp.tile([66, S], BF16, tag=f"kaug{h}", name=f"kaug{h}") for h in range(HPC)]

            # ---- phase 1: projections ----
            # qT/kT: [e-chunk 128 (2 heads), s] via lhsT=w*T chunk, rhs=xT
            for (w_sb, b_sb, dest) in ((wq_sb, bq_sb, qaug), (wk_sb, bk_sb, kaug)):
                for ec in range(2):
                    for st in range(S // 512):
                        pt = ps.tile([128, 512], F32, tag="big", name="pt_proj")
                        for dc in range(NDC):
                            nc.tensor.matmul(
                                pt[:, :],
                                w_sb[:, dc, ec * 128:(ec + 1) * 128],
                                xT_sb[:, dc, st * 512:(st + 1) * 512],
                                start=(dc == 0), stop=(dc == NDC - 1),
                            )
                        stg = wp.tile([128, 512], BF16, tag="stg", name="stg_proj")
                        nc.vector.tensor_scalar_add(stg, pt[:, :], b_sb[:, ec:ec + 1])
                        # partition-shift halves into per-head tensors via DMA
                        sl = slice(st * 512, (st + 1) * 512)
                        nc.sync.dma_start(out=dest[2 * ec][0:64, sl], in_=stg[0:64, :])
                        nc.sync.dma_start(out=dest[2 * ec + 1][0:64, sl], in_=stg[64:128, :])
            # v natural: [s-tile 128, 256]
            v_sb = pp.tile([128, NQT, EC], BF16, tag="v_sb")
            for st in range(NQT):
                pt = ps.tile([128, 512], F32, tag="big", name="pt_vproj")
                for dc in range(NDC):
                    nc.tensor.matmul(
                        pt[:, 0:EC],
                        xT_sb[:, dc, st * 128:(st + 1) * 128],
                        wv_sb[:, dc, :],
                        start=(dc == 0), stop=(dc == NDC - 1),
                    )
                nc.vector.tensor_add(v_sb[:, st, :], pt[:, 0:EC], bv_sb)

            # ones rows of kaug
            for h in range(HPC):
                nc.vector.memset(kaug[h][64:66, :], 1.0)

            # attn_outT chunks [hd-dims 128 (2 heads), s]
            aoT = [pp.tile([128, S], BF16, tag=f"aoT{c}", name=f"aoT{c}") for c in range(2)]

            # ---- phase 2 per head ----
            def emit_natural(h):
                sums_z = sp.tile([128, NQT], F32, tag="sums", name=f"sums_{h}")
                recip_z = sp.tile([128, NQT], F32, tag="recip", name=f"recip_{h}")
                # --- natural side ---
                for qt in range(NQT):
                    kw = (qt + 1) * 128
                    nkh = (kw + 1023) // 1024
                    e_row = wp.tile([128, S], BF16, tag="e_row", name=f"e_{h}_{qt}")
                    acc4 = sp.tile([128, 4], F32, tag="acc4", name=f"acc_{h}_{qt}")
                    for kh in range(nkh):
                        cur = min(1024, kw - kh * 1024)
                        s_ps = ps.tile([128, 1024], F32, tag="big", name=f"sps_{h}_{qt}_{kh}")
                        for kb in range((cur + 511) // 512):
                            n = min(512, cur - kb * 512)
                            nc.tensor.matmul(
                                s_ps[:, kb * 512:kb * 512 + n],
                                qaug[h][0:64, qt * 128:(qt + 1) * 128],
                                kaug[h][0:64, kh * 1024 + kb * 512:kh * 1024 + kb * 512 + n],
                                start=True, stop=True,
                            )
                        dg = qt * 128 - kh * 1024  # diagonal subtile offset in this half
                        if 0 <= dg < 1024:
                            nc.vector.tensor_add(s_ps[:, dg:dg + 128],
                                                 s_ps[:, dg:dg + 128], mN_sb)
                        acc_ap = sums_z[:, qt:qt + 1] if nkh == 1 else acc4[:, kh:kh + 1]
                        nc.scalar.activation(
                            e_row[:, kh * 1024:kh * 1024 + cur], s_ps[:, 0:cur],
                            AF.Exp, accum_out=acc_ap,
                        )
                    if nkh > 1:
                        nc.vector.tensor_reduce(
                            sums_z[:, qt:qt + 1], acc4[:, 0:nkh],
                            axis=mybir.AxisListType.X, op=mybir.AluOpType.add,
                        )
                    nc.vector.reciprocal(recip_z[:, qt:qt + 1], sums_z[:, qt:qt + 1])
                    w_row = wp.tile([128, S], BF16, tag="w_row", name=f"w_{h}_{qt}")
                    nc.vector.tensor_scalar_mul(w_row[:, 0:kw], e_row[:, 0:kw],
                                                recip_z[:, qt:qt + 1])
                    nc.sync.dma_start(
                        out=attnw[h, qt * 128:(qt + 1) * 128, 0:kw],
                        in_=w_row[:, 0:kw],
                    )
                    yield

                # --- -lnZ rows (hi + residual) into qaug[h][64:66] ---
                lnzn = sp.tile([128, NQT], F32, tag="lnzn", name=f"lnzn_{h}")
                nc.scalar.activation(lnzn, recip_z, AF.Ln)   # ln(1/Z) = -lnZ
                hi_bf = sp.tile([128, NQT], BF16, tag="hi_bf", name=f"hibf_{h}")
                nc.vector.tensor_copy(hi_bf, lnzn)
                hi_f = sp.tile([128, NQT], F32, tag="hi_f", name=f"hif_{h}")
                nc.vector.tensor_copy(hi_f, hi_bf)
                res_bf = sp.tile([128, NQT], BF16, tag="res_bf", name=f"resbf_{h}")
                nc.vector.tensor_sub(res_bf, lnzn, hi_f)
                for src, row in ((hi_bf, 64), (res_bf, 65)):
                    ptt = psa.tile([16, 128], BF16, tag="attn", name=f"ptt_{h}_{row}")
                    nc.tensor.transpose(ptt, src, id_sb)
                    stt = sp.tile([16, 128], BF16, tag="stt", name=f"stt_{h}_{row}")
                    nc.vector.tensor_copy(stt, ptt)
                    nc.sync.dma_start(out=qaug[h][row:row + 1, :], in_=stt[:, :])
                yield

            def emit_transposed(h):
                # --- transposed side + attn@v (512-grid-aligned blocks) ---
                hc, hp = h // 2, h % 2
                for qh in range(2):
                    q0 = qh * 1024
                    pa = [psa.tile([128, 512], F32, tag="attn", name=f"pa_{h}_{qh}_{j}")
                          for j in range(2)]
                    ktmax = q0 // 128 + 8
                    for kt in range(ktmax):
                        qlo = max(q0, kt * 128)
                        n = q0 + 1024 - qlo
                        sT = ps.tile([128, 1024], F32, tag="big", name=f"sT_{h}_{qh}_{kt}")
                        for qb in range((n + 511) // 512):
                            nn_ = min(512, n - qb * 512)
                            nc.tensor.matmul(
                                sT[:, qb * 512:qb * 512 + nn_],
                                kaug[h][:, kt * 128:(kt + 1) * 128],
                                qaug[h][:, qlo + qb * 512:qlo + qb * 512 + nn_],
                                start=True, stop=True,
                            )
                        if kt * 128 >= q0:  # diagonal subtile at offset 0
                            nc.vector.tensor_add(sT[:, 0:128], sT[:, 0:128], mT_sb)
                        wt = wtp.tile([128, 1024], BF16, tag="wt", name=f"wt_{h}_{qh}_{kt}")
                        nc.scalar.activation(wt[:, 0:n], sT[:, 0:n], AF.Exp)
                        for j in range(2):
                            blo = q0 + 512 * j
                            if kt * 128 >= blo + 512:
                                continue
                            lo = max(blo, qlo)
                            ps_off = lo - blo
                            wt_off = lo - qlo
                            wdt = blo + 512 - lo
                            last_kt = min(ktmax - 1, (blo + 511) // 128)
                            nc.tensor.matmul(
                                pa[j][hp * 64:(hp + 1) * 64, ps_off:ps_off + wdt],
                                v_sb[:, kt, h * 64:(h + 1) * 64],
                                wt[:, wt_off:wt_off + wdt],
                                start=(kt == 0), stop=(kt == last_kt),
                                tile_position=(0, hp * 64),
                            )
                        yield
                    for j in range(2):
                        nc.vector.tensor_copy(
                            aoT[hc][hp * 64:(hp + 1) * 64,
                                    q0 + 512 * j:q0 + 512 * (j + 1)],
                            pa[j][hp * 64:(hp + 1) * 64, :],
                        )

            # Fine-grained interleave: natural(h) zipped with transposed(h-1)
            # so the PE stream always has independent matmuls to fill the
            # bubbles left by the ACT-gated softmax pipelines (keeps HAM warm).
            def _drain(g):
                for _ in g:
                    pass

            prev = None
            for h in range(HPC):
                cur = emit_natural(h)
                if prev is None:
                    _drain(cur)
                else:
                    gens = [cur, prev]
                    while gens:
                        for g in list(gens):
                            try:
                                next(g)
                            except StopIteration:
                                gens.remove(g)
                prev = emit_transposed(h)
            _drain(prev)

            if _DEBUG:
                aodbg = nc.dram_tensor("aodbg", [2, 128, S], BF16, kind="ExternalOutput")
                vdbg = nc.dram_tensor("vdbg", [128, NQT * EC], BF16, kind="ExternalOutput")
                for c in range(2):
                    nc.sync.dma_start(out=aodbg[c, :, :], in_=aoT[c][:, :])
                nc.sync.dma_start(out=vdbg[:, :], in_=v_sb[:, :, :])

            # ---- phase 3: out-projection partial ----
            for st in range(NQT):
                o_sb = wp.tile([128, 1024], F32, tag="o_sb", name=f"osb_{st}")
                for eb in range(2):
                    po = ps.tile([128, 512], F32, tag="big", name=f"po_{st}_{eb}")
                    for c in range(2):
                        nc.tensor.matmul(
                            po[:, :],
                            aoT[c][:, st * 128:(st + 1) * 128],
                            wo_sb[:, c, eb * 512:(eb + 1) * 512],
                            start=(c == 0), stop=(c == 1),
                        )
                    nc.vector.tensor_copy(o_sb[:, eb * 512:(eb + 1) * 512], po)
                nc.sync.dma_start(out=outp[st * 128:(st + 1) * 128, :], in_=o_sb)

    return nc


_NC = None


def _get_nc():
    global _NC
    if _NC is None:
        _NC = _build_nc()
    return _NC


# ---------------------------------------------------------------------------
# Host wrapper
# ---------------------------------------------------------------------------
def _prep_core_inputs(c, x, Wq, bq, Wk, bk, Wv, bv, Wo, bo, masks):
    b = c // 4
    sl = slice((c % 4) * HPC * HD, (c % 4) * HPC * HD + EC)
    bf = ml_dtypes.bfloat16
    sc = 1.0 / np.sqrt(HD)
    xT = np.ascontiguousarray(x[b].T).astype(bf)
    wqT = np.ascontiguousarray((Wq[sl] * sc).T).astype(bf)
    wkT = np.ascontiguousarray(Wk[sl].T).astype(bf)
    wvT = np.ascontiguousarray(Wv[sl].T).astype(bf)
    woT = np.ascontiguousarray(Wo[:, sl].T).astype(bf)
    bq2 = np.ascontiguousarray((bq[sl] * sc).reshape(2, 128).T).astype(np.float32)
    bk2 = np.ascontiguousarray(bk[sl].reshape(2, 128).T).astype(np.float32)
    bvr = np.broadcast_to(bv[sl], (128, EC)).astype(bf)
    mN, mT, ident = masks
    return {
        "xT": xT, "wqT": wqT, "wkT": wkT, "wvT": wvT, "woT": woT,
        "bq2": bq2, "bk2": bk2, "bvr": bvr,
        "mskN": mN, "mskT": mT, "ident": ident,
    }


def _ensure_ntff_hook():
    """Install an antenv.axon_hooks shim (missing from this image) so
    run_bass_kernel_spmd(trace=True) can capture NTFF profiles via the
    axon PJRT .so — mirrors trn_agent_boot._ntff_profile_via_ctypes."""
    import sys as _sys, types, contextlib as _ctx, ctypes
    try:
        from antenv.axon_hooks import get_axon_ntff_profile_hook  # noqa: F401
        return True
    except ImportError:
        pass
    so_path = "/opt/axon/libaxon_pjrt.so"
    if not os.path.exists(so_path):
        return False
    lib = ctypes.CDLL(so_path)
    if not hasattr(lib, "axon_start_nrt_profile"):
        return False
    lib.axon_start_nrt_profile.argtypes = [ctypes.POINTER(ctypes.c_int64), ctypes.c_size_t]
    lib.axon_start_nrt_profile.restype = ctypes.c_int64
    lib.axon_stop_nrt_profile.argtypes = [ctypes.c_char_p]
    lib.axon_stop_nrt_profile.restype = ctypes.c_int64

    @_ctx.contextmanager
    def _hook(output_dir, device_ids):
        import jax
        jax.devices()
        if device_ids:
            ids = (ctypes.c_int64 * len(device_ids))(*device_ids)
            rc = lib.axon_start_nrt_profile(ids, len(device_ids))
        else:
            rc = lib.axon_start_nrt_profile(None, 0)
        if rc != 0:
            raise RuntimeError(f"axon_start_nrt_profile rc={rc}")
        try:
            yield
        finally:
            n = lib.axon_stop_nrt_profile(str(output_dir).encode())
            print(f"ntff profile: {n} file(s) -> {output_dir}")

    import antenv
    mod = types.ModuleType("antenv.axon_hooks")
    mod.get_axon_ntff_profile_hook = lambda: _hook
    mod.set_axon_ntff_profile_hook = lambda h: None
    antenv.axon_hooks = mod
    _sys.modules["antenv.axon_hooks"] = mod
    # artifacts upload needs a bucket; keep everything local instead
    bass_utils.upload_artifacts = lambda tmpdir: tmpdir
    return True


def _run(inputs, trace=False):
    if trace:
        _ensure_ntff_hook()
    nc = _get_nc()
    x = np.asarray(inputs["x"], np.float32)
    args = (x, np.asarray(inputs["Wq"], np.float32), np.asarray(inputs["bq"], np.float32),
            np.asarray(inputs["Wk"], np.float32), np.asarray(inputs["bk"], np.float32),
            np.asarray(inputs["Wv"], np.float32), np.asarray(inputs["bv"], np.float32),
            np.asarray(inputs["Wo"], np.float32), np.asarray(inputs["bo"], np.float32))
    bo = args[8]

    ii, jj = np.meshgrid(np.arange(128), np.arange(128), indexing="ij")
    mN = np.where(jj <= ii, 0.0, MASK_VAL).astype(np.float32)
    mT = mN.T.copy()
    ident = np.eye(128, dtype=ml_dtypes.bfloat16)
    masks = (mN, mT, ident)

    in_maps = [_prep_core_inputs(c, *args, masks) for c in range(NCORES)]
    res = bass_utils.run_bass_kernel_spmd(
        nc, in_maps, core_ids=list(range(NCORES)), trace=trace,
    )

    attn = np.zeros((B, H, S, S), dtype=np.float32)
    out = np.zeros((B, S, D), dtype=np.float32)
    for c in range(NCORES):
        b = c // 4
        h0 = (c % 4) * HPC
        attn[b, h0:h0 + HPC] = np.asarray(res.results[c]["attnw"]).astype(np.float32)
        out[b] += np.asarray(res.results[c]["outp"]).astype(np.float32)
    out += bo.astype(np.float32)
    return (out, attn), res


def kernel(**inputs):
    (out, attn), _ = _run(inputs, trace=False)
    return (out, attn)


def kernel_traced(**inputs):
    """Like kernel() but also returns BassKernelResults (exec_time_ns etc.)."""
    return _run(inputs, trace=True)


# revision 23
# speedup vs baseline: 1.1890x; 1.0683x over previous
"""Causal multi-head attention (B=2, S=2048, D=1024, H=16, hd=64) for 8 Trainium2
NeuronCores, returning (output, attn_weights) like torch nn.MultiheadAttention.

Sharding: core c handles batch b=c//4 and 4 heads (c%4)*4..+4 (data+tensor
parallel per the Megatron hint). Each core computes its heads' causal attention
weights (written bf16, upper triangle left to the runtime's zero-init) and a
partial output projection; the host sums partials and adds biases.

Device-side dataflow per head:
  natural side : scores = qT.T @ kT (PE, bf16, K=64) -> +mask on diagonal tile
                 -> exp on ACT with accum_out row-sums -> w = e * (1/Z) on DVE
                 -> DMA causal rows to HBM
  transposed   : scoresT = kT_aug.T @ qT_aug with two extra contraction rows
                 carrying ones * (-lnZ) (hi + bf16 residual), so exp gives the
                 *normalized* wT directly -> attn_outT += v.T-stationary matmuls
                 -> out-projection partial from attn_outT.
"""

import numpy as np
import ml_dtypes

import concourse.bass as bass
import concourse.mybir as mybir
import concourse.tile as tile
from concourse import bass_utils
from concourse.vector_clock import ScopedClock

BF16 = mybir.dt.bfloat16
F32 = mybir.dt.float32
AF = mybir.ActivationFunctionType

import os
_DEBUG = bool(os.environ.get("KERNEL_DEBUG"))

B, S, D, H = 2, 2048, 1024, 16
HD = D // H            # 64
HPC = 4                # heads per core
EC = HPC * HD          # 256 e-dims per core
NCORES = 8
NQT = S // 128         # 16 q row-tiles
MASK_VAL = -30000.0


# ---------------------------------------------------------------------------
# Container-walrus workaround: CTRL/Drain instructions only support one
# sync-wait slot; Tile's kernel-tail drain carries one wait per DMA-HW queue.
# Split extras onto single-wait NOPs before the all-engine barrier.
# ---------------------------------------------------------------------------
_patched = False


def _patch_tile_drain():
    global _patched
    if _patched:
        return
    _patched = True

    orig_add = tile.TileContext._add_instruction

    def _add_instruction(self, inst):
        si = getattr(inst, "sync_info", None)
        eng = getattr(inst, "engine", None)
        if (
            si is not None
            and si.on_wait
            and len(si.on_wait) > 1
            and eng is not None
            and eng != mybir.EngineType.Unassigned
        ):
            waits = list(si.on_wait)
            si.on_wait = [waits[-1]]
            nc = self.nc
            for w in waits[:-1]:
                nop = mybir.InstNoOp(
                    name=f"I-waitsplit-{nc.next_id()}",
                    sync_info=mybir.SyncInfo(on_wait=[w], on_update=[]),
                    engine=eng,
                    bass_nofuse=True,
                )
                orig_add(self, nop)
        orig_add(self, inst)

    tile.TileContext._add_instruction = _add_instruction

    def _drain_and_barrier(self, tick_clock, wait_clock):
        nc = self.nc
        drain_inst = nc.sync.drain()
        wait_clock.add_sem_waits(
            drain_inst.ins, ScopedClock({None: tick_clock.global_clock})
        )
        si = drain_inst.ins.sync_info
        if si is not None and si.on_wait and len(si.on_wait) > 1:
            waits = list(si.on_wait)
            si.on_wait = [waits[0]]
            for w in waits[1:]:
                nop = mybir.InstNoOp(
                    name=f"I-waitsplit-{nc.next_id()}",
                    sync_info=mybir.SyncInfo(on_wait=[w], on_update=[]),
                    engine=drain_inst.ins.engine,
                    bass_nofuse=True,
                )
                nc.register_instruction(nop, overwrite=True)
                nc.cur_bb.bb.add_instruction(nop)

        nc.all_engine_barrier()
        assert self.sems is not None
        popped = nc._tile_sem_poison_stack.pop()
        assert popped is self._sem_poison
        nc.clear_and_free_semaphores(list(self.sems.allocated().values()))
        nc.all_engine_barrier()

    tile.TileContext._drain_and_barrier = _drain_and_barrier


# ---------------------------------------------------------------------------
# Kernel build
# ---------------------------------------------------------------------------
def _build_nc():
    _patch_tile_drain()
    nc = bass.Bass("TRN2")

    # ---- I/O ----
    xT = nc.dram_tensor("xT", [D, S], BF16, kind="ExternalInput")          # x[b].T
    wqT = nc.dram_tensor("wqT", [D, EC], BF16, kind="ExternalInput")       # (Wq/8).T slice
    wkT = nc.dram_tensor("wkT", [D, EC], BF16, kind="ExternalInput")
    wvT = nc.dram_tensor("wvT", [D, EC], BF16, kind="ExternalInput")
    woT = nc.dram_tensor("woT", [EC, D], BF16, kind="ExternalInput")       # Wo[:, sl].T
    bq2 = nc.dram_tensor("bq2", [128, 2], F32, kind="ExternalInput")       # per-chunk bias
    bk2 = nc.dram_tensor("bk2", [128, 2], F32, kind="ExternalInput")
    bvr = nc.dram_tensor("bvr", [128, EC], BF16, kind="ExternalInput")     # bv replicated
    mskN = nc.dram_tensor("mskN", [128, 128], F32, kind="ExternalInput")   # natural diag mask
    mskT = nc.dram_tensor("mskT", [128, 128], F32, kind="ExternalInput")   # transposed diag mask
    ident = nc.dram_tensor("ident", [128, 128], BF16, kind="ExternalInput")

    attnw = nc.dram_tensor("attnw", [HPC, S, S], BF16, kind="ExternalOutput")
    outp = nc.dram_tensor("outp", [S, D], F32, kind="ExternalOutput")

    NDC = D // 128  # 8 contraction chunks

    with tile.TileContext(nc) as tc:
        with (
            tc.tile_pool(name="persist", bufs=1) as pp,
            tc.tile_pool(name="work", bufs=3) as wp,
            tc.tile_pool(name="wtp", bufs=3) as wtp,
            tc.tile_pool(name="small", bufs=4) as sp,
            tc.tile_pool(name="ps", bufs=3, space="PSUM") as ps,
            tc.tile_pool(name="psa", bufs=2, space="PSUM") as psa,
        ):
            # ---- phase 0: load inputs (one big DMA per tensor; SP trigger
            # generation is serial and ~600ns each, so fewer is faster) ----
            wq_sb = pp.tile([128, NDC, EC], BF16, tag="wq_sb")
            wk_sb = pp.tile([128, NDC, EC], BF16, tag="wk_sb")
            wv_sb = pp.tile([128, NDC, EC], BF16, tag="wv_sb")
            for dst, src in ((wq_sb, wqT), (wk_sb, wkT), (wv_sb, wvT)):
                nc.sync.dma_start(
                    out=dst, in_=src[:, :].rearrange("(c p) e -> p c e", p=128))
            xT_sb = pp.tile([128, NDC, S], BF16, tag="xT_sb")
            nc.sync.dma_start(
                out=xT_sb, in_=xT[:, :].rearrange("(c p) s -> p c s", p=128))
            wo_sb = pp.tile([128, 2, D], BF16, tag="wo_sb")
            nc.sync.dma_start(
                out=wo_sb, in_=woT[:, :].rearrange("(c p) e -> p c e", p=128))
            bq_sb = pp.tile([128, 2], F32, tag="bq_sb")
            bk_sb = pp.tile([128, 2], F32, tag="bk_sb")
            bv_sb = pp.tile([128, EC], BF16, tag="bv_sb")
            mN_sb = pp.tile([128, 128], F32, tag="mN_sb")
            mT_sb = pp.tile([128, 128], F32, tag="mT_sb")
            id_sb = pp.tile([128, 128], BF16, tag="id_sb")
            nc.sync.dma_start(out=bq_sb, in_=bq2[:, :])
            nc.sync.dma_start(out=bk_sb, in_=bk2[:, :])
            nc.sync.dma_start(out=bv_sb, in_=bvr[:, :])
            nc.sync.dma_start(out=mN_sb, in_=mskN[:, :])
            nc.sync.dma_start(out=mT_sb, in_=mskT[:, :])
            nc.sync.dma_start(out=id_sb, in_=ident[:, :])

            # ---- per-head q/k tensors with 2 aug rows ----
            qaug = [pp.tile([66, S], BF16, tag=f"qaug{h}", name=f"qaug{h}") for h in range(HPC)]
            kaug = [pp.tile([66, S], BF16, tag=f"kaug{h}", name=f"kaug{h}") for h in range(HPC)]

            # ---- phase 1: projections ----
            # qT/kT: [e-chunk 128 (2 heads), s] via lhsT=w*T chunk, rhs=xT,
            # staged in chunked layout then partition-shifted per head in one
            # DMA per (head) to the aug tensors.
            qc = [pp.tile([128, S], BF16, tag=f"qc{c}", name=f"qc{c}") for c in range(2)]
            kc = [pp.tile([128, S], BF16, tag=f"kc{c}", name=f"kc{c}") for c in range(2)]
            for (w_sb, b_sb, chunks, dest) in (
                (wq_sb, bq_sb, qc, qaug), (wk_sb, bk_sb, kc, kaug)):
                for ec in range(2):
                    for st in range(S // 512):
                        pt = ps.tile([128, 512], F32, tag="big", name="pt_proj")
                        for dc in range(NDC):
                            nc.tensor.matmul(
                                pt[:, :],
                                w_sb[:, dc, ec * 128:(ec + 1) * 128],
                                xT_sb[:, dc, st * 512:(st + 1) * 512],
                                start=(dc == 0), stop=(dc == NDC - 1),
                            )
                        nc.vector.tensor_scalar_add(
                            chunks[ec][:, st * 512:(st + 1) * 512],
                            pt[:, :], b_sb[:, ec:ec + 1])
                    nc.sync.dma_start(out=dest[2 * ec][0:64, :], in_=chunks[ec][0:64, :])
                    nc.sync.dma_start(out=dest[2 * ec + 1][0:64, :], in_=chunks[ec][64:128, :])
            # v natural: [s-tile 128, 256]
            v_sb = pp.tile([128, NQT, EC], BF16, tag="v_sb")
            for st in range(NQT):
                pt = ps.tile([128, 512], F32, tag="big", name="pt_vproj")
                for dc in range(NDC):
                    nc.tensor.matmul(
                        pt[:, 0:EC],
                        xT_sb[:, dc, st * 128:(st + 1) * 128],
                        wv_sb[:, dc, :],
                        start=(dc == 0), stop=(dc == NDC - 1),
                    )
                nc.vector.tensor_add(v_sb[:, st, :], pt[:, 0:EC], bv_sb)

            # ones rows of kaug
            for h in range(HPC):
                nc.vector.memset(kaug[h][64:66, :], 1.0)

            # attn_outT chunks [hd-dims 128 (2 heads), s]
            aoT = [pp.tile([128, S], BF16, tag=f"aoT{c}", name=f"aoT{c}") for c in range(2)]

            # ---- phase 2 per head ----
            def emit_natural(h):
                sums_z = sp.tile([128, NQT], F32, tag="sums", name=f"sums_{h}")
                recip_z = sp.tile([128, NQT], F32, tag="recip", name=f"recip_{h}")
                # --- natural side ---
                for qt in range(NQT):
                    kw = (qt + 1) * 128
                    nkh = (kw + 1023) // 1024
                    e_row = wp.tile([128, S], BF16, tag="e_row", name=f"e_{h}_{qt}")
                    acc4 = sp.tile([128, 4], F32, tag="acc4", name=f"acc_{h}_{qt}")
                    for kh in range(nkh):
                        cur = min(1024, kw - kh * 1024)
                        s_ps = ps.tile([128, 1024], F32, tag="big", name=f"sps_{h}_{qt}_{kh}")
                        for kb in range((cur + 511) // 512):
                            n = min(512, cur - kb * 512)
                            nc.tensor.matmul(
                                s_ps[:, kb * 512:kb * 512 + n],
                                qaug[h][0:64, qt * 128:(qt + 1) * 128],
                                kaug[h][0:64, kh * 1024 + kb * 512:kh * 1024 + kb * 512 + n],
                                start=True, stop=True,
                            )
                        dg = qt * 128 - kh * 1024  # diagonal subtile offset in this half
                        if 0 <= dg < 1024:
                            nc.vector.tensor_add(s_ps[:, dg:dg + 128],
                                                 s_ps[:, dg:dg + 128], mN_sb)
                        acc_ap = sums_z[:, qt:qt + 1] if nkh == 1 else acc4[:, kh:kh + 1]
                        nc.scalar.activation(
                            e_row[:, kh * 1024:kh * 1024 + cur], s_ps[:, 0:cur],
                            AF.Exp, accum_out=acc_ap,
                        )
                    if nkh > 1:
                        nc.vector.tensor_reduce(
                            sums_z[:, qt:qt + 1], acc4[:, 0:nkh],
                            axis=mybir.AxisListType.X, op=mybir.AluOpType.add,
                        )
                    nc.vector.reciprocal(recip_z[:, qt:qt + 1], sums_z[:, qt:qt + 1])
                    w_row = wp.tile([128, S], BF16, tag="w_row", name=f"w_{h}_{qt}")
                    nc.vector.tensor_scalar_mul(w_row[:, 0:kw], e_row[:, 0:kw],
                                                recip_z[:, qt:qt + 1])
                    nc.sync.dma_start(
                        out=attnw[h, qt * 128:(qt + 1) * 128, 0:kw],
                        in_=w_row[:, 0:kw],
                    )
                    yield

                # --- -lnZ rows (hi + residual) into qaug[h][64:66] ---
                lnzn = sp.tile([128, NQT], F32, tag="lnzn", name=f"lnzn_{h}")
                nc.scalar.activation(lnzn, recip_z, AF.Ln)   # ln(1/Z) = -lnZ
                hi_bf = sp.tile([128, NQT], BF16, tag="hi_bf", name=f"hibf_{h}")
                nc.vector.tensor_copy(hi_bf, lnzn)
                hi_f = sp.tile([128, NQT], F32, tag="hi_f", name=f"hif_{h}")
                nc.vector.tensor_copy(hi_f, hi_bf)
                res_bf = sp.tile([128, NQT], BF16, tag="res_bf", name=f"resbf_{h}")
                nc.vector.tensor_sub(res_bf, lnzn, hi_f)
                for src, row in ((hi_bf, 64), (res_bf, 65)):
                    ptt = psa.tile([16, 128], BF16, tag="attn", name=f"ptt_{h}_{row}")
                    nc.tensor.transpose(ptt, src, id_sb)
                    stt = sp.tile([16, 128], BF16, tag="stt", name=f"stt_{h}_{row}")
                    nc.vector.tensor_copy(stt, ptt)
                    nc.sync.dma_start(out=qaug[h][row:row + 1, :], in_=stt[:, :])
                yield

            def emit_transposed(h):
                # --- transposed side + attn@v (512-grid-aligned blocks) ---
                hc, hp = h // 2, h % 2
                for qh in range(2):
                    q0 = qh * 1024
                    pa = [psa.tile([128, 512], F32, tag="attn", name=f"pa_{h}_{qh}_{j}")
                          for j in range(2)]
                    ktmax = q0 // 128 + 8
                    for kt in range(ktmax):
                        qlo = max(q0, kt * 128)
                        n = q0 + 1024 - qlo
                        sT = ps.tile([128, 1024], F32, tag="big", name=f"sT_{h}_{qh}_{kt}")
                        for qb in range((n + 511) // 512):
                            nn_ = min(512, n - qb * 512)
                            nc.tensor.matmul(
                                sT[:, qb * 512:qb * 512 + nn_],
                                kaug[h][:, kt * 128:(kt + 1) * 128],
                                qaug[h][:, qlo + qb * 512:qlo + qb * 512 + nn_],
                                start=True, stop=True,
                            )
                        if kt * 128 >= q0:  # diagonal subtile at offset 0
                            nc.vector.tensor_add(sT[:, 0:128], sT[:, 0:128], mT_sb)
                        wt = wtp.tile([128, 1024], BF16, tag="wt", name=f"wt_{h}_{qh}_{kt}")
                        nc.scalar.activation(wt[:, 0:n], sT[:, 0:n], AF.Exp)
                        for j in range(2):
                            blo = q0 + 512 * j
                            if kt * 128 >= blo + 512:
                                continue
                            lo = max(blo, qlo)
                            ps_off = lo - blo
                            wt_off = lo - qlo
                            wdt = blo + 512 - lo
                            last_kt = min(ktmax - 1, (blo + 511) // 128)
                            nc.tensor.matmul(
                                pa[j][hp * 64:(hp + 1) * 64, ps_off:ps_off + wdt],
                                v_sb[:, kt, h * 64:(h + 1) * 64],
                                wt[:, wt_off:wt_off + wdt],
                                start=(kt == 0), stop=(kt == last_kt),
                                tile_position=(0, hp * 64),
                            )
                        yield
                    for j in range(2):
                        nc.vector.tensor_copy(
                            aoT[hc][hp * 64:(hp + 1) * 64,
                                    q0 + 512 * j:q0 + 512 * (j + 1)],
                            pa[j][hp * 64:(hp + 1) * 64, :],
                        )

            # Fine-grained interleave: natural(h) zipped with transposed(h-1)
            # so the PE stream always has independent matmuls to fill the
            # bubbles left by the ACT-gated softmax pipelines (keeps HAM warm).
            def _drain(g):
                for _ in g:
                    pass

            prev = None
            for h in range(HPC):
                cur = emit_natural(h)
                if prev is None:
                    _drain(cur)
                else:
                    gens = [cur, prev]
                    while gens:
                        for g in list(gens):
                            try:
                                next(g)
                            except StopIteration:
                                gens.remove(g)
                prev = emit_transposed(h)
            _drain(prev)

            if _DEBUG:
                aodbg = nc.dram_tensor("aodbg", [2, 128, S], BF16, kind="ExternalOutput")
                vdbg = nc.dram_tensor("vdbg", [128, NQT * EC], BF16, kind="ExternalOutput")
                for c in range(2):
                    nc.sync.dma_start(out=aodbg[c, :, :], in_=aoT[c][:, :])
                nc.sync.dma_start(out=vdbg[:, :], in_=v_sb[:, :, :])

            # ---- phase 3: out-projection partial ----
            for st in range(NQT):
                o_sb = wp.tile([128, 1024], F32, tag="o_sb", name=f"osb_{st}")
                for eb in range(2):
                    po = ps.tile([128, 512], F32, tag="big", name=f"po_{st}_{eb}")
                    for c in range(2):
                        nc.tensor.matmul(
                            po[:, :],
                            aoT[c][:, st * 128:(st + 1) * 128],
                            wo_sb[:, c, eb * 512:(eb + 1) * 512],
                            start=(c == 0), stop=(c == 1),
                        )
                    nc.vector.tensor_copy(o_sb[:, eb * 512:(eb + 1) * 512], po)
                nc.sync.dma_start(out=outp[st * 128:(st + 1) * 128, :], in_=o_sb)

    return nc


_NC = None


def _get_nc():
    global _NC
    if _NC is None:
        _NC = _build_nc()
    return _NC


# ---------------------------------------------------------------------------
# Host wrapper
# ---------------------------------------------------------------------------
def _prep_core_inputs(c, x, Wq, bq, Wk, bk, Wv, bv, Wo, bo, masks):
    b = c // 4
    sl = slice((c % 4) * HPC * HD, (c % 4) * HPC * HD + EC)
    bf = ml_dtypes.bfloat16
    sc = 1.0 / np.sqrt(HD)
    xT = np.ascontiguousarray(x[b].T).astype(bf)
    wqT = np.ascontiguousarray((Wq[sl] * sc).T).astype(bf)
    wkT = np.ascontiguousarray(Wk[sl].T).astype(bf)
    wvT = np.ascontiguousarray(Wv[sl].T).astype(bf)
    woT = np.ascontiguousarray(Wo[:, sl].T).astype(bf)
    bq2 = np.ascontiguousarray((bq[sl] * sc).reshape(2, 128).T).astype(np.float32)
    bk2 = np.ascontiguousarray(bk[sl].reshape(2, 128).T).astype(np.float32)
    bvr = np.broadcast_to(bv[sl], (128, EC)).astype(bf)
    mN, mT, ident = masks
    return {
        "xT": xT, "wqT": wqT, "wkT": wkT, "wvT": wvT, "woT": woT,
        "bq2": bq2, "bk2": bk2, "bvr": bvr,
        "mskN": mN, "mskT": mT, "ident": ident,
    }


def _ensure_ntff_hook():
    """Install an antenv.axon_hooks shim (missing from this image) so
    run_bass_kernel_spmd(trace=True) can capture NTFF profiles via the
    axon PJRT .so — mirrors trn_agent_boot._ntff_profile_via_ctypes."""
    import sys as _sys, types, contextlib as _ctx, ctypes
    try:
        from antenv.axon_hooks import get_axon_ntff_profile_hook  # noqa: F401
        return True
    except ImportError:
        pass
    so_path = "/opt/axon/libaxon_pjrt.so"
    if not os.path.exists(so_path):
        return False
    lib = ctypes.CDLL(so_path)
    if not hasattr(lib, "axon_start_nrt_profile"):
        return False
    lib.axon_start_nrt_profile.argtypes = [ctypes.POINTER(ctypes.c_int64), ctypes.c_size_t]
    lib.axon_start_nrt_profile.restype = ctypes.c_int64
    lib.axon_stop_nrt_profile.argtypes = [ctypes.c_char_p]
    lib.axon_stop_nrt_profile.restype = ctypes.c_int64

    @_ctx.contextmanager
    def _hook(output_dir, device_ids):
        import jax
        jax.devices()
        if device_ids:
            ids = (ctypes.c_int64 * len(device_ids))(*device_ids)
            rc = lib.axon_start_nrt_profile(ids, len(device_ids))
        else:
            rc = lib.axon_start_nrt_profile(None, 0)
        if rc != 0:
            raise RuntimeError(f"axon_start_nrt_profile rc={rc}")
        try:
            yield
        finally:
            n = lib.axon_stop_nrt_profile(str(output_dir).encode())
            print(f"ntff profile: {n} file(s) -> {output_dir}")

    import antenv
    mod = types.ModuleType("antenv.axon_hooks")
    mod.get_axon_ntff_profile_hook = lambda: _hook
    mod.set_axon_ntff_profile_hook = lambda h: None
    antenv.axon_hooks = mod
    _sys.modules["antenv.axon_hooks"] = mod
    # artifacts upload needs a bucket; keep everything local instead
    bass_utils.upload_artifacts = lambda tmpdir: tmpdir
    return True


def _run(inputs, trace=False):
    if trace:
        _ensure_ntff_hook()
    nc = _get_nc()
    x = np.asarray(inputs["x"], np.float32)
    args = (x, np.asarray(inputs["Wq"], np.float32), np.asarray(inputs["bq"], np.float32),
            np.asarray(inputs["Wk"], np.float32), np.asarray(inputs["bk"], np.float32),
            np.asarray(inputs["Wv"], np.float32), np.asarray(inputs["bv"], np.float32),
            np.asarray(inputs["Wo"], np.float32), np.asarray(inputs["bo"], np.float32))
    bo = args[8]

    ii, jj = np.meshgrid(np.arange(128), np.arange(128), indexing="ij")
    mN = np.where(jj <= ii, 0.0, MASK_VAL).astype(np.float32)
    mT = mN.T.copy()
    ident = np.eye(128, dtype=ml_dtypes.bfloat16)
    masks = (mN, mT, ident)

    in_maps = [_prep_core_inputs(c, *args, masks) for c in range(NCORES)]
    res = bass_utils.run_bass_kernel_spmd(
        nc, in_maps, core_ids=list(range(NCORES)), trace=trace,
    )

    attn = np.zeros((B, H, S, S), dtype=np.float32)
    out = np.zeros((B, S, D), dtype=np.float32)
    for c in range(NCORES):
        b = c // 4
        h0 = (c % 4) * HPC
        attn[b, h0:h0 + HPC] = np.asarray(res.results[c]["attnw"]).astype(np.float32)
        out[b] += np.asarray(res.results[c]["outp"]).astype(np.float32)
    out += bo.astype(np.float32)
    return (out, attn), res


def kernel(**inputs):
    (out, attn), _ = _run(inputs, trace=False)
    return (out, attn)


def kernel_traced(**inputs):
    """Like kernel() but also returns BassKernelResults (exec_time_ns etc.)."""
    return _run(inputs, trace=True)
